# revision 7
# baseline (speedup 1.0000x reference)
"""Trainium2 Bass kernel for nn_AttentionSequence (DIN-style attention, 8 cores).

Data-parallel over batch (2048 -> 8 x 256). Per core, rows are processed in
s-major order (row r = s*256 + b) so the per-batch query term enters as a
fixed [80, 256] matrix add.

Math (per core shard, R = 256*200 = 51200 rows):
  xb = W1f^T mov + U           (mm1 on PE; U-add + PSUM evac on DVE; fp16 stash)
  Sxb2 = sum xb^2              (ACT square + accum_out)
  AR1: global sum of Sxb2; mean1 is host-precomputed (linear in inputs)
  p1 = sigmoid(s1*xb + t1)     (ACT), h1 = xb*p1 (GPSIMD)  [alpha1=0 fast path]
  x2 = W2p^T h1                (mm2, pairs packed at partitions 0:64/64:104)
  stats2 via bn_stats/bn_aggr  (DVE), AR2
  p2 = sigmoid(s2*x2+t2), h2 = x2*p2
  score[s,b] via mm3 (stationary h2-slice [40,128], moving Wp) -> PSUM [128b, 200s]
  softmax over s (max-subtract, exp with accum sum)
  out[e,b] = sum_s w[s,b]*keys[b,s,e] via per-batch 2-matmul einsum
"""
import numpy as np

import concourse.bacc as bacc
import concourse.tile as tile
import concourse.mybir as mybir
from concourse.bass_utils import run_bass_kernel_spmd

F16 = mybir.dt.float16
F32 = mybir.dt.float32
AF = mybir.ActivationFunctionType
OP = mybir.AluOpType

M = 8
B, S, E = 2048, 200, 64
H1, H2 = 80, 40
BSH = B // M            # 256 batches per core
R = BSH * S             # 51200 rows per core
CH = 512                # chunk rows (= 2 s-values x 256 batches)
NCH = R // CH           # 100 chunks
NPAIR = NCH // 2        # 50 chunk pairs
RP = R // 2             # 25600 stash cols for packed x2
EPS = 1e-5
NTOT = float(B * S)

SQ_SL = 2048            # ACT square slice
NSQ = R // SQ_SL        # 25
SG_SL = 2048            # sigmoid1/gp-mul slice
NSG = R // SG_SL        # 25
C_SL = 2048             # phase C slice over RP
NC_SL = RP // C_SL      # 12.5 -> handle remainder
KNB = 16                # kn batches per streamed block

_CACHE = {}


def _build(alpha1_nz, alpha2_nz, b2_nz):
    nc = bacc.Bacc()

    mov_d = nc.declare_dram_parameter("mov", [128, R], F16, isOutput=False)
    w1f_d = nc.declare_dram_parameter("w1f", [128, H1], F16, isOutput=False)
    u_d = nc.declare_dram_parameter("u", [H1, BSH], F32, isOutput=False)
    mean1_d = nc.declare_dram_parameter("mean1", [H1, 1], F32, isOutput=False)
    g1_d = nc.declare_dram_parameter("g1", [H1, 1], F32, isOutput=False)
    be1_d = nc.declare_dram_parameter("be1", [H1, 1], F32, isOutput=False)
    am1_d = nc.declare_dram_parameter("am1", [H1, 2], F32, isOutput=False)
    w2p_d = nc.declare_dram_parameter("w2p", [H1, 64], F16, isOutput=False)
    g2_d = nc.declare_dram_parameter("g2", [104, 1], F32, isOutput=False)
    be2_d = nc.declare_dram_parameter("be2", [104, 1], F32, isOutput=False)
    am2_d = nc.declare_dram_parameter("am2", [104, 2], F32, isOutput=False)
    b2c_d = nc.declare_dram_parameter("b2c", [104, 1], F32, isOutput=False)
    wp_d = nc.declare_dram_parameter("wp", [104, 1], F16, isOutput=False)
    kn1_d = nc.declare_dram_parameter("kn1", [128, BSH * 64], F16, isOutput=False)
    kn2_d = nc.declare_dram_parameter("kn2", [72, BSH * 64], F16, isOutput=False)
    iden_d = nc.declare_dram_parameter("iden", [128, 128], F16, isOutput=False)

    out_d = nc.declare_dram_parameter("out", [64, BSH], F32, isOutput=True)

    ar1_in = nc.dram_tensor("ar1_in", [H1, 1], F32)
    ar1_out = nc.dram_tensor("ar1_out", [H1, 1], F32, addr_space="Shared")
    ar2_in = nc.dram_tensor("ar2_in", [104, 2], F32)
    ar2_out = nc.dram_tensor("ar2_out", [104, 2], F32, addr_space="Shared")

    with tile.TileContext(nc) as tc:
        with (
            tc.tile_pool(name="const", bufs=1) as cp,
            tc.tile_pool(name="stash", bufs=1) as stp,
            tc.tile_pool(name="work", bufs=2) as wp_pool,
            tc.tile_pool(name="movr", bufs=4) as movr,
            tc.tile_pool(name="stats", bufs=1) as sp,
        ):
            # ---- constants ----
            w1f = cp.tile([128, H1], F16)
            nc.sync.dma_start(w1f[:], w1f_d[:, :])
            u2 = cp.tile([H1, CH], F32)
            nc.sync.dma_start(u2[:, 0:BSH], u_d[:, :])
            nc.sync.dma_start(u2[:, BSH:CH], u_d[:, :])
            w2p = cp.tile([H1, 64], F16)
            nc.sync.dma_start(w2p[:], w2p_d[:, :])
            wpc = cp.tile([104, 1], F16)
            nc.sync.dma_start(wpc[:], wp_d[:, :])
            iden = cp.tile([128, 128], F16)
            nc.sync.dma_start(iden[:], iden_d[:, :])
            mean1 = sp.tile([H1, 1], F32)
            nc.sync.dma_start(mean1[:], mean1_d[:, :])
            g1 = sp.tile([H1, 1], F32)
            nc.sync.dma_start(g1[:], g1_d[:, :])
            be1 = sp.tile([H1, 1], F32)
            nc.sync.dma_start(be1[:], be1_d[:, :])
            g2 = sp.tile([104, 1], F32)
            nc.sync.dma_start(g2[:], g2_d[:, :])
            be2 = sp.tile([104, 1], F32)
            nc.sync.dma_start(be2[:], be2_d[:, :])
            if alpha1_nz:
                am1 = sp.tile([H1, 2], F32)
                nc.sync.dma_start(am1[:], am1_d[:, :])
            if alpha2_nz:
                am2 = sp.tile([104, 2], F32)
                nc.sync.dma_start(am2[:], am2_d[:, :])
            if b2_nz:
                b2c = sp.tile([104, 1], F32)
                nc.sync.dma_start(b2c[:], b2c_d[:, :])

            # ---- big stashes ----
            xb = stp.tile([H1, R], F16)          # layer-1 pre-BN activations
            x2s = stp.tile([104, RP], F16)       # packed layer-2 pre-BN
            sqcols = sp.tile([H1, 32], F32)      # ACT square accum partials
            nc.vector.memset(sqcols[:], 0.0)
            epsc = sp.tile([104, 1], F32)
            nc.vector.memset(epsc[:], EPS)
            bns = sp.tile([104, 6 * NPAIR], F32)  # bn_stats partials

            # ================= Phase A =================
            with tc.tile_pool(name="psA", bufs=4, space="PSUM") as psA:
                for j in range(NCH):
                    mv = movr.tile([128, CH], F16, name="mv")
                    nc.sync.dma_start(mv[:], mov_d[:, j * CH:(j + 1) * CH])
                    x1p = psA.tile([H1, CH], F32, name="x1p")
                    nc.tensor.matmul(x1p[:], w1f[:], mv[:], start=True, stop=True)
                    # evac + U add (fp32 psum + fp32 U -> fp16 stash)
                    nc.vector.tensor_tensor(
                        xb[:, j * CH:(j + 1) * CH], x1p[:], u2[:], op=OP.add)
                    if (j + 1) % 4 == 0:
                        k = j // 4
                        sqj = wp_pool.tile([H1, SQ_SL], F16, name="sqj", tag="actout")
                        nc.scalar.activation(
                            sqj[:], xb[:, k * SQ_SL:(k + 1) * SQ_SL], AF.Square,
                            accum_out=sqcols[:, k:k + 1])

            # ---- AR1: global sum of squares ----
            sx2g = sp.tile([H1, 1], F32)
            nc.vector.tensor_reduce(sx2g[:], sqcols[:], axis=mybir.AxisListType.X,
                                    op=OP.add)
            nc.sync.dma_start(ar1_in[:, :], sx2g[:])
            nc.gpsimd.collective_compute(
                "AllReduce", OP.add, replica_groups=[list(range(M))],
                ins=[ar1_in[:, :]], outs=[ar1_out[:, :]])
            sx2a = sp.tile([H1, 1], F32)
            nc.sync.dma_start(sx2a[:], ar1_out[:, :])

            # stats1: var = E[x^2] - mean1^2 ; s1 = g1/sqrt(var+eps) ; t1 = be1 - mean1*s1
            ex2 = sp.tile([H1, 1], F32)
            nc.vector.tensor_scalar(ex2[:], sx2a[:], 1.0 / NTOT, None, OP.mult)
            msq = sp.tile([H1, 1], F32)
            nc.vector.tensor_tensor(msq[:], mean1[:], mean1[:], op=OP.mult)
            var1 = sp.tile([H1, 1], F32)
            nc.vector.tensor_tensor(var1[:], ex2[:], msq[:], op=OP.subtract)
            sd1 = sp.tile([H1, 1], F32)
            nc.scalar.activation(sd1[:], var1[:], AF.Sqrt, bias=epsc[0:H1, 0:1], scale=1.0)
            rsd1 = sp.tile([H1, 1], F32)
            nc.vector.reciprocal(rsd1[:], sd1[:])
            s1 = sp.tile([H1, 1], F32)
            nc.vector.tensor_tensor(s1[:], g1[:], rsd1[:], op=OP.mult)
            ms1 = sp.tile([H1, 1], F32)
            nc.vector.tensor_tensor(ms1[:], mean1[:], s1[:], op=OP.mult)
            t1 = sp.tile([H1, 1], F32)
            nc.vector.tensor_tensor(t1[:], be1[:], ms1[:], op=OP.subtract)

            # ================= Phase B =================
            zc = sp.tile([104, 1], F32)
            nc.vector.memset(zc[:], 0.0)
            with tc.tile_pool(name="psB", bufs=3, space="PSUM") as psB:
                for blk in range(NSG):          # 25 blocks of 2048 rows = 2 pairs
                    sl = slice(blk * SG_SL, (blk + 1) * SG_SL)
                    p1 = wp_pool.tile([H1, SG_SL], F16, name="p1", tag="actout")
                    nc.scalar.activation(p1[:], xb[:, sl], AF.Sigmoid,
                                         bias=t1[:, 0:1], scale=s1[:, 0:1])
                    if alpha1_nz:
                        nc.vector.tensor_scalar(p1[:], p1[:], am1[:, 0:1],
                                                am1[:, 1:2], OP.mult, OP.add)
                    h1 = wp_pool.tile([H1, SG_SL], F16, name="h1", tag="gpout", bufs=3)
                    nc.gpsimd.tensor_tensor(h1[:], xb[:, sl], p1[:], op=OP.mult)
                    for pp in range(2):         # 2 chunk-pairs per block
                        p = blk * 2 + pp
                        x2p = psB.tile([104, CH], F32, name="x2p")
                        c0 = pp * 2 * CH
                        nc.tensor.matmul(x2p[0:64, :], w2p[:],
                                         h1[:, c0:c0 + CH], start=True, stop=True)
                        nc.tensor.matmul(x2p[64:104, :], w2p[:, 0:H2],
                                         h1[:, c0 + CH:c0 + 2 * CH], start=True,
                                         stop=True, tile_position=(0, 64))
                        nc.vector.tensor_tensor(
                            x2s[:, p * CH:(p + 1) * CH], x2p[:],
                            zc[:].broadcast_to([104, CH]), op=OP.add)
                        nc.vector.bn_stats(bns[:, p * 6:(p + 1) * 6],
                                           x2s[:, p * CH:(p + 1) * CH])

            # ---- AR2: layer-2 stats ----
            bna = sp.tile([104, 2], F32)
            nc.vector.bn_aggr(bna[:], bns[:])
            # convert (mean, var) -> (sum, sumsq) scaled by local count R
            s2s = sp.tile([104, 2], F32)
            nc.vector.tensor_scalar(s2s[:, 0:1], bna[:, 0:1], float(R), None, OP.mult)
            m2sq = sp.tile([104, 1], F32)
            nc.vector.tensor_tensor(m2sq[:], bna[:, 0:1], bna[:, 0:1], op=OP.mult)
            nc.vector.tensor_tensor(m2sq[:], bna[:, 1:2], m2sq[:], op=OP.add)
            nc.vector.tensor_scalar(s2s[:, 1:2], m2sq[:], float(R), None, OP.mult)
            nc.sync.dma_start(ar2_in[:, :], s2s[:])
            nc.gpsimd.collective_compute(
                "AllReduce", OP.add, replica_groups=[list(range(M))],
                ins=[ar2_in[:, :]], outs=[ar2_out[:, :]])
            s2a = sp.tile([104, 2], F32)
            nc.sync.dma_start(s2a[:], ar2_out[:, :])

            mean2 = sp.tile([104, 1], F32)
            nc.vector.tensor_scalar(mean2[:], s2a[:, 0:1], 1.0 / NTOT, None, OP.mult)
            ex22 = sp.tile([104, 1], F32)
            nc.vector.tensor_scalar(ex22[:], s2a[:, 1:2], 1.0 / NTOT, None, OP.mult)
            msq2 = sp.tile([104, 1], F32)
            nc.vector.tensor_tensor(msq2[:], mean2[:], mean2[:], op=OP.mult)
            var2 = sp.tile([104, 1], F32)
            nc.vector.tensor_tensor(var2[:], ex22[:], msq2[:], op=OP.subtract)
            sd2 = sp.tile([104, 1], F32)
            nc.scalar.activation(sd2[:], var2[:], AF.Sqrt, bias=epsc[:, 0:1], scale=1.0)
            rsd2 = sp.tile([104, 1], F32)
            nc.vector.reciprocal(rsd2[:], sd2[:])
            s2 = sp.tile([104, 1], F32)
            nc.vector.tensor_tensor(s2[:], g2[:], rsd2[:], op=OP.mult)
            ms2 = sp.tile([104, 1], F32)
            nc.vector.tensor_tensor(ms2[:], mean2[:], s2[:], op=OP.mult)
            t2 = sp.tile([104, 1], F32)
            nc.vector.tensor_tensor(t2[:], be2[:], ms2[:], op=OP.subtract)

            # ================= Phase C =================
            with (
                tc.tile_pool(name="psScore", bufs=1, space="PSUM") as psS,
                tc.tile_pool(name="psT", bufs=1, space="PSUM") as psT,
                tc.tile_pool(name="psOut", bufs=1, space="PSUM") as psO,
                tc.tile_pool(name="knr", bufs=3) as knr,
                tc.tile_pool(name="smx", bufs=2) as smx,
            ):
                # h2' slices (sigmoid2 + gate mul), stream into ring
                h2ring = []
                nco = 0
                while nco < RP:
                    w_sl = min(C_SL, RP - nco)
                    sl = slice(nco, nco + w_sl)
                    p2 = wp_pool.tile([104, C_SL], F16, name="p2", tag="actout")
                    nc.scalar.activation(p2[:, 0:w_sl], x2s[:, sl], AF.Sigmoid,
                                         bias=t2[:, 0:1], scale=s2[:, 0:1])
                    if alpha2_nz:
                        nc.vector.tensor_scalar(p2[:, 0:w_sl], p2[:, 0:w_sl],
                                                am2[:, 0:1], am2[:, 1:2],
                                                OP.mult, OP.add)
                    if b2_nz:
                        nc.vector.tensor_scalar(x2s[:, sl], x2s[:, sl],
                                                b2c[:, 0:1], None, OP.add)
                    h2 = wp_pool.tile([104, C_SL], F16, name="h2", tag="gpout", bufs=3)
                    nc.gpsimd.tensor_tensor(h2[:, 0:w_sl], x2s[:, sl], p2[:, 0:w_sl],
                                            op=OP.mult)
                    h2ring.append((nco, w_sl, h2))
                    nco += w_sl

                def h2_slice(col, width):
                    for base, w_sl, t in h2ring:
                        if base <= col and col + width <= base + w_sl:
                            return t[:, col - base:col - base + width]
                    raise AssertionError("h2 slice spans tiles")

                score_ps = [psS.tile([128, 200], F32, name=f"score{g}")
                            for g in range(2)]
                # mm3: score[s, b-group] columns
                for s in range(S):
                    pgrp, sl4 = divmod(s, 4)
                    if sl4 < 2:
                        rows, cbase = slice(0, H2), pgrp * CH + sl4 * BSH
                        tp = (0, 0)
                        wslice = wpc[0:H2, :]
                    else:
                        rows, cbase = slice(64, 104), pgrp * CH + (sl4 - 2) * BSH
                        tp = (64, 0)
                        wslice = wpc[64:104, :]
                    for g in range(2):
                        st = h2_slice(cbase + g * 128, 128)[rows, :]
                        nc.tensor.matmul(score_ps[g][:, s:s + 1], st, wslice,
                                         start=True, stop=True, tile_position=tp)

                outp = psO.tile([64, BSH], F32)
                outs = smx.tile([64, BSH], F32, name="outs", bufs=1)
                for g in range(2):
                    # softmax over s for 128 batches
                    nmx = smx.tile([128, 1], F32, name="nmx")
                    nc.vector.tensor_reduce(nmx[:], score_ps[g][:], op=OP.max,
                                            axis=mybir.AxisListType.X, negate=True)
                    ex = smx.tile([128, 200], F32, name="ex")
                    se = smx.tile([128, 1], F32, name="se")
                    nc.scalar.activation(ex[:], score_ps[g][:], AF.Exp,
                                         bias=nmx[:, 0:1], scale=1.0,
                                         accum_out=se[:, 0:1])
                    rse = smx.tile([128, 1], F32, name="rse")
                    nc.vector.reciprocal(rse[:], se[:])
                    wgt = smx.tile([128, 200], F16, name="wgt")
                    nc.vector.tensor_scalar(wgt[:], ex[:], rse[:, 0:1], None, OP.mult)
                    # transpose w -> [s, b]
                    wta_p = psT.tile([128, 128], F16, name="wta_p")
                    nc.tensor.transpose(wta_p[:], wgt[:, 0:128], iden[:])
                    wtb_p = psT.tile([72, 128], F16, name="wtb_p")
                    nc.tensor.transpose(wtb_p[:], wgt[:, 128:200], iden[:])
                    wta = smx.tile([128, 128], F16, name="wta")
                    nc.scalar.copy(wta[:], wta_p[:])
                    wtb = smx.tile([72, 128], F16, name="wtb")
                    nc.scalar.copy(wtb[:], wtb_p[:])
                    # einsum per batch
                    for bb in range(0, 128, KNB):
                        kt1 = knr.tile([128, KNB * 64], F16, name="kt1")
                        gb = g * 128 + bb
                        nc.sync.dma_start(kt1[:], kn1_d[:, gb * 64:(gb + KNB) * 64])
                        kt2 = knr.tile([72, KNB * 64], F16, name="kt2")
                        nc.sync.dma_start(kt2[:], kn2_d[:, gb * 64:(gb + KNB) * 64])
                        for bi in range(KNB):
                            bcol = g * 128 + bb + bi
                            nc.tensor.matmul(
                                outp[:, bcol:bcol + 1],
                                kt1[:, bi * 64:(bi + 1) * 64],
                                wta[:, bb + bi:bb + bi + 1],
                                start=True, stop=False)
                            nc.tensor.matmul(
                                outp[:, bcol:bcol + 1],
                                kt2[:, bi * 64:(bi + 1) * 64],
                                wtb[:, bb + bi:bb + bi + 1],
                                start=False, stop=True)
                    nc.scalar.copy(outs[:, g * 128:(g + 1) * 128],
                                   outp[:, g * 128:(g + 1) * 128])
                nc.sync.dma_start(out_d[:, :], outs[:])

    nc.compile()
    return nc


def _prep_inputs(query, keys, W1, b1, gamma1, beta1, alpha1,
                 W2, b2, gamma2, beta2, alpha2, Wp, bp):
    f32 = np.float32
    query = np.asarray(query, f32)
    keys = np.asarray(keys, f32)
    W1 = np.asarray(W1, f32); b1 = np.asarray(b1, f32)
    W2 = np.asarray(W2, f32); b2 = np.asarray(b2, f32)
    Wp = np.asarray(Wp, f32)

    W1a, W1b, W1c, W1d = W1[0:64], W1[64:128], W1[128:192], W1[192:256]
    w1f = np.concatenate([W1b - W1c, W1d], axis=0).astype(np.float16)  # [128, 80]

    q2 = query[:, 0, :]                                  # [B, 64]
    # global mean of xb (exact, fp32)
    mk = keys.reshape(-1, E).mean(0)                     # [64]
    mqk = (keys * query).reshape(-1, E).mean(0)          # [64]
    mu_u = (q2 @ (W1a + W1c) + b1).mean(0)               # [80]
    mean1 = ((W1b - W1c).T @ mk + W1d.T @ mqk + mu_u).astype(f32)

    w2p = np.zeros((H1, 64), np.float16)
    w2p[:, 0:H2] = W2.astype(np.float16)

    wp104 = np.zeros((104, 1), np.float16)
    wp104[0:H2, 0] = Wp[:, 0].astype(np.float16)
    wp104[64:104, 0] = Wp[:, 0].astype(np.float16)

    def pad104(v, fill):
        out = np.full((104, 1), fill, f32)
        out[0:H2, 0] = v
        out[64:104, 0] = v
        return out

    g2c = pad104(np.asarray(gamma2, f32), 1.0)
    be2c = pad104(np.asarray(beta2, f32), 0.0)
    b2c = pad104(b2, 0.0)
    am2 = np.concatenate([pad104(1.0 - np.asarray(alpha2, f32), 1.0),
                          pad104(np.asarray(alpha2, f32), 0.0)], axis=1)
    am1 = np.stack([1.0 - np.asarray(alpha1, f32), np.asarray(alpha1, f32)],
                   axis=1).astype(f32)

    iden = np.eye(128, dtype=np.float16)

    in_maps = []
    for m in range(M):
        bm = slice(m * BSH, (m + 1) * BSH)
        k_sh = keys[bm]                                  # [256, 200, 64]
        q_sh = q2[bm]                                    # [256, 64]
        kT = np.ascontiguousarray(k_sh.transpose(2, 1, 0).reshape(E, R))
        qkT = np.ascontiguousarray(
            (k_sh * q_sh[:, None, :]).transpose(2, 1, 0).reshape(E, R))
        mov = np.concatenate([kT, qkT], axis=0).astype(np.float16)
        u = np.ascontiguousarray((q_sh @ (W1a + W1c) + b1).T).astype(f32)
        ks = k_sh.transpose(1, 0, 2)                     # [200, 256, 64]
        kn1 = np.ascontiguousarray(ks[0:128].reshape(128, BSH * 64)).astype(np.float16)
        kn2 = np.ascontiguousarray(ks[128:200].reshape(72, BSH * 64)).astype(np.float16)
        in_maps.append(dict(
            mov=mov, w1f=w1f, u=u, mean1=mean1.reshape(H1, 1),
            g1=np.asarray(gamma1, f32).reshape(H1, 1),
            be1=np.asarray(beta1, f32).reshape(H1, 1),
            am1=am1, w2p=w2p, g2=g2c, be2=be2c, am2=am2, b2c=b2c,
            wp=wp104, kn1=kn1, kn2=kn2, iden=iden,
        ))
    flags = (bool(np.any(np.asarray(alpha1))), bool(np.any(np.asarray(alpha2))),
             bool(np.any(np.asarray(b2))))
    return in_maps, flags


def kernel(**inputs):
    in_maps, flags = _prep_inputs(**inputs)
    if flags not in _CACHE:
        _CACHE[flags] = _build(*flags)
    nc = _CACHE[flags]
    res = run_bass_kernel_spmd(nc, in_maps, core_ids=list(range(M)))
    outs = [res.results[m]["out"].T for m in range(M)]   # [256, 64] each
    return np.concatenate(outs, axis=0).astype(np.float32)


if __name__ == "__main__":
    rng = np.random.default_rng(0)
    pass


# revision 8
# speedup vs baseline: 1.1547x; 1.1547x over previous
"""Trainium2 Bass kernel for nn_AttentionSequence (DIN-style attention, 8 cores).

Data-parallel over batch (2048 -> 8 x 256). Per core, rows are processed in
s-major order (row r = s*256 + b) so the per-batch query term enters as a
fixed [80, 256] matrix add.

Math (per core shard, R = 256*200 = 51200 rows):
  xb = W1f^T mov + U           (mm1 on PE; U-add + PSUM evac on DVE; fp16 stash)
  Sxb2 = sum xb^2              (ACT square + accum_out)
  AR1: global sum of Sxb2; mean1 is host-precomputed (linear in inputs)
  p1 = sigmoid(s1*xb + t1)     (ACT), h1 = xb*p1 (GPSIMD)  [alpha1=0 fast path]
  x2 = W2p^T h1                (mm2, pairs packed at partitions 0:64/64:104)
  stats2 via bn_stats/bn_aggr  (DVE), AR2
  p2 = sigmoid(s2*x2+t2), h2 = x2*p2
  score[s,b] via mm3 (stationary h2-slice [40,128], moving Wp) -> PSUM [128b, 200s]
  softmax over s (max-subtract, exp with accum sum)
  out[e,b] = sum_s w[s,b]*keys[b,s,e] via per-batch 2-matmul einsum
"""
import numpy as np

import concourse.bacc as bacc
import concourse.tile as tile
import concourse.mybir as mybir
from concourse.bass_utils import run_bass_kernel_spmd

F16 = mybir.dt.float16
F32 = mybir.dt.float32
AF = mybir.ActivationFunctionType
OP = mybir.AluOpType

M = 8
B, S, E = 2048, 200, 64
H1, H2 = 80, 40
BSH = B // M            # 256 batches per core
R = BSH * S             # 51200 rows per core
CH = 512                # chunk rows (= 2 s-values x 256 batches)
NCH = R // CH           # 100 chunks
NPAIR = NCH // 2        # 50 chunk pairs
RP = R // 2             # 25600 stash cols for packed x2
EPS = 1e-5
NTOT = float(B * S)

SQ_SL = 2048            # ACT square slice
NSQ = R // SQ_SL        # 25
SG_SL = 2048            # sigmoid1/gp-mul slice
NSG = R // SG_SL        # 25
C_SL = 2048             # phase C slice over RP
NC_SL = RP // C_SL      # 12.5 -> handle remainder
KNB = 16                # kn batches per streamed block

_CACHE = {}


def _build(alpha1_nz, alpha2_nz, b2_nz):
    nc = bacc.Bacc()

    mov_d = nc.declare_dram_parameter("mov", [128, R], F16, isOutput=False)
    w1f_d = nc.declare_dram_parameter("w1f", [128, H1], F16, isOutput=False)
    u_d = nc.declare_dram_parameter("u", [H1, BSH], F32, isOutput=False)
    mean1_d = nc.declare_dram_parameter("mean1", [H1, 1], F32, isOutput=False)
    g1_d = nc.declare_dram_parameter("g1", [H1, 1], F32, isOutput=False)
    be1_d = nc.declare_dram_parameter("be1", [H1, 1], F32, isOutput=False)
    am1_d = nc.declare_dram_parameter("am1", [H1, 2], F32, isOutput=False)
    w2p_d = nc.declare_dram_parameter("w2p", [H1, 64], F16, isOutput=False)
    g2_d = nc.declare_dram_parameter("g2", [104, 1], F32, isOutput=False)
    be2_d = nc.declare_dram_parameter("be2", [104, 1], F32, isOutput=False)
    am2_d = nc.declare_dram_parameter("am2", [104, 2], F32, isOutput=False)
    b2c_d = nc.declare_dram_parameter("b2c", [104, 1], F32, isOutput=False)
    wp_d = nc.declare_dram_parameter("wp", [104, 1], F16, isOutput=False)
    wp2c_d = nc.declare_dram_parameter("wp2c", [104, 2], F16, isOutput=False)
    kn1_d = nc.declare_dram_parameter("kn1", [128, BSH * 64], F16, isOutput=False)
    kn2_d = nc.declare_dram_parameter("kn2", [72, BSH * 64], F16, isOutput=False)
    iden_d = nc.declare_dram_parameter("iden", [128, 128], F16, isOutput=False)

    out_d = nc.declare_dram_parameter("out", [64, BSH], F32, isOutput=True)

    ar1_in = nc.dram_tensor("ar1_in", [H1, 1], F32)
    ar1_out = nc.dram_tensor("ar1_out", [H1, 1], F32, addr_space="Shared")
    ar2_in = nc.dram_tensor("ar2_in", [104, 2], F32)
    ar2_out = nc.dram_tensor("ar2_out", [104, 2], F32, addr_space="Shared")

    with tile.TileContext(nc) as tc:
        with (
            tc.tile_pool(name="const", bufs=1) as cp,
            tc.tile_pool(name="stash", bufs=1) as stp,
            tc.tile_pool(name="work", bufs=2) as wp_pool,
            tc.tile_pool(name="movr", bufs=4) as movr,
            tc.tile_pool(name="stats", bufs=1) as sp,
        ):
            # ---- constants ----
            w1f = cp.tile([128, H1], F16)
            nc.sync.dma_start(w1f[:], w1f_d[:, :])
            u4 = cp.tile([H1, 2 * CH], F32)
            for _r in range(4):
                nc.sync.dma_start(u4[:, _r * BSH:(_r + 1) * BSH], u_d[:, :])
            w2p = cp.tile([H1, 64], F16)
            nc.sync.dma_start(w2p[:], w2p_d[:, :])
            wp2c = cp.tile([104, 2], F16)
            nc.sync.dma_start(wp2c[:], wp2c_d[:, :])
            iden = cp.tile([128, 128], F16)
            nc.sync.dma_start(iden[:], iden_d[:, :])
            mean1 = sp.tile([H1, 1], F32)
            nc.sync.dma_start(mean1[:], mean1_d[:, :])
            g1 = sp.tile([H1, 1], F32)
            nc.sync.dma_start(g1[:], g1_d[:, :])
            be1 = sp.tile([H1, 1], F32)
            nc.sync.dma_start(be1[:], be1_d[:, :])
            g2 = sp.tile([104, 1], F32)
            nc.sync.dma_start(g2[:], g2_d[:, :])
            be2 = sp.tile([104, 1], F32)
            nc.sync.dma_start(be2[:], be2_d[:, :])
            if alpha1_nz:
                am1 = sp.tile([H1, 2], F32)
                nc.sync.dma_start(am1[:], am1_d[:, :])
            if alpha2_nz:
                am2 = sp.tile([104, 2], F32)
                nc.sync.dma_start(am2[:], am2_d[:, :])
            if b2_nz:
                b2c = sp.tile([104, 1], F32)
                nc.sync.dma_start(b2c[:], b2c_d[:, :])

            # ---- big stashes ----
            xb = stp.tile([H1, R], F16)          # layer-1 pre-BN activations
            x2s = stp.tile([104, RP], F16)       # packed layer-2 pre-BN
            sqcols = sp.tile([H1, 32], F32)      # ACT square accum partials
            nc.vector.memset(sqcols[:], 0.0)
            epsc = sp.tile([104, 1], F32)
            nc.vector.memset(epsc[:], EPS)
            bns = sp.tile([104, 6 * NPAIR], F32)  # bn_stats partials

            # ================= Phase A =================
            with tc.tile_pool(name="psA", bufs=2, space="PSUM") as psA:
                for j2 in range(NCH // 2):
                    x1p = psA.tile([H1, 2 * CH], F32, name="x1p")
                    for k2 in range(2):
                        j = j2 * 2 + k2
                        mv = movr.tile([128, CH], F16, name="mv")
                        nc.sync.dma_start(mv[:], mov_d[:, j * CH:(j + 1) * CH])
                        nc.tensor.matmul(x1p[:, k2 * CH:(k2 + 1) * CH], w1f[:],
                                         mv[:], start=True, stop=True)
                    # evac + U add (fp32 psum + fp32 U -> fp16 stash)
                    nc.vector.tensor_tensor(
                        xb[:, j2 * 2 * CH:(j2 + 1) * 2 * CH], x1p[:], u4[:],
                        op=OP.add)
                    if j2 % 2 == 1:
                        k = j2 // 2
                        sqj = wp_pool.tile([H1, SQ_SL], F16, name="sqj", tag="actout")
                        nc.scalar.activation(
                            sqj[:], xb[:, k * SQ_SL:(k + 1) * SQ_SL], AF.Square,
                            accum_out=sqcols[:, k:k + 1])

            # ---- AR1: global sum of squares ----
            sx2g = sp.tile([H1, 1], F32)
            nc.vector.tensor_reduce(sx2g[:], sqcols[:], axis=mybir.AxisListType.X,
                                    op=OP.add)
            nc.sync.dma_start(ar1_in[:, :], sx2g[:])
            nc.gpsimd.collective_compute(
                "AllReduce", OP.add, replica_groups=[list(range(M))],
                ins=[ar1_in[:, :]], outs=[ar1_out[:, :]])
            sx2a = sp.tile([H1, 1], F32)
            nc.sync.dma_start(sx2a[:], ar1_out[:, :])

            # stats1: var = E[x^2] - mean1^2 ; s1 = g1/sqrt(var+eps) ; t1 = be1 - mean1*s1
            ex2 = sp.tile([H1, 1], F32)
            nc.vector.tensor_scalar(ex2[:], sx2a[:], 1.0 / NTOT, None, OP.mult)
            msq = sp.tile([H1, 1], F32)
            nc.vector.tensor_tensor(msq[:], mean1[:], mean1[:], op=OP.mult)
            var1 = sp.tile([H1, 1], F32)
            nc.vector.tensor_tensor(var1[:], ex2[:], msq[:], op=OP.subtract)
            sd1 = sp.tile([H1, 1], F32)
            nc.scalar.activation(sd1[:], var1[:], AF.Sqrt, bias=epsc[0:H1, 0:1], scale=1.0)
            rsd1 = sp.tile([H1, 1], F32)
            nc.vector.reciprocal(rsd1[:], sd1[:])
            s1 = sp.tile([H1, 1], F32)
            nc.vector.tensor_tensor(s1[:], g1[:], rsd1[:], op=OP.mult)
            ms1 = sp.tile([H1, 1], F32)
            nc.vector.tensor_tensor(ms1[:], mean1[:], s1[:], op=OP.mult)
            t1 = sp.tile([H1, 1], F32)
            nc.vector.tensor_tensor(t1[:], be1[:], ms1[:], op=OP.subtract)

            # ================= Phase B =================
            zc = sp.tile([104, 1], F32)
            nc.vector.memset(zc[:], 0.0)
            with tc.tile_pool(name="psB", bufs=3, space="PSUM") as psB:
                for blk in range(NSG):          # 25 blocks of 2048 rows = 2 pairs
                    sl = slice(blk * SG_SL, (blk + 1) * SG_SL)
                    p1 = wp_pool.tile([H1, SG_SL], F16, name="p1", tag="actout")
                    nc.scalar.activation(p1[:], xb[:, sl], AF.Sigmoid,
                                         bias=t1[:, 0:1], scale=s1[:, 0:1])
                    if alpha1_nz:
                        nc.vector.tensor_scalar(p1[:], p1[:], am1[:, 0:1],
                                                am1[:, 1:2], OP.mult, OP.add)
                    h1 = wp_pool.tile([H1, SG_SL], F16, name="h1", tag="gpout", bufs=3)
                    SPL = 1408
                    nc.vector.tensor_tensor(
                        h1[:, 0:SPL], xb[:, blk * SG_SL:blk * SG_SL + SPL],
                        p1[:, 0:SPL], op=OP.mult)
                    nc.gpsimd.tensor_tensor(
                        h1[:, SPL:SG_SL], xb[:, blk * SG_SL + SPL:(blk + 1) * SG_SL],
                        p1[:, SPL:SG_SL], op=OP.mult)
                    for pp in range(2):         # 2 chunk-pairs per block
                        p = blk * 2 + pp
                        x2p = psB.tile([104, CH], F32, name="x2p")
                        c0 = pp * 2 * CH
                        nc.tensor.matmul(x2p[0:64, :], w2p[:],
                                         h1[:, c0:c0 + CH], start=True, stop=True)
                        nc.tensor.matmul(x2p[64:104, :], w2p[:, 0:H2],
                                         h1[:, c0 + CH:c0 + 2 * CH], start=True,
                                         stop=True, tile_position=(0, 64))
                        if pp == 0:
                            nc.scalar.copy(x2s[:, p * CH:(p + 1) * CH], x2p[:])
                        else:
                            nc.vector.tensor_tensor(
                                x2s[:, p * CH:(p + 1) * CH], x2p[:],
                                zc[:].broadcast_to([104, CH]), op=OP.add)
                        nc.vector.bn_stats(bns[:, p * 6:(p + 1) * 6],
                                           x2s[:, p * CH:(p + 1) * CH])

            # ---- AR2: layer-2 stats ----
            bna = sp.tile([104, 2], F32)
            nc.vector.bn_aggr(bna[:], bns[:])
            # convert (mean, var) -> (sum, sumsq) scaled by local count R
            s2s = sp.tile([104, 2], F32)
            nc.vector.tensor_scalar(s2s[:, 0:1], bna[:, 0:1], float(R), None, OP.mult)
            m2sq = sp.tile([104, 1], F32)
            nc.vector.tensor_tensor(m2sq[:], bna[:, 0:1], bna[:, 0:1], op=OP.mult)
            nc.vector.tensor_tensor(m2sq[:], bna[:, 1:2], m2sq[:], op=OP.add)
            nc.vector.tensor_scalar(s2s[:, 1:2], m2sq[:], float(R), None, OP.mult)
            nc.sync.dma_start(ar2_in[:, :], s2s[:])
            nc.gpsimd.collective_compute(
                "AllReduce", OP.add, replica_groups=[list(range(M))],
                ins=[ar2_in[:, :]], outs=[ar2_out[:, :]])
            s2a = sp.tile([104, 2], F32)
            nc.sync.dma_start(s2a[:], ar2_out[:, :])

            mean2 = sp.tile([104, 1], F32)
            nc.vector.tensor_scalar(mean2[:], s2a[:, 0:1], 1.0 / NTOT, None, OP.mult)
            ex22 = sp.tile([104, 1], F32)
            nc.vector.tensor_scalar(ex22[:], s2a[:, 1:2], 1.0 / NTOT, None, OP.mult)
            msq2 = sp.tile([104, 1], F32)
            nc.vector.tensor_tensor(msq2[:], mean2[:], mean2[:], op=OP.mult)
            var2 = sp.tile([104, 1], F32)
            nc.vector.tensor_tensor(var2[:], ex22[:], msq2[:], op=OP.subtract)
            sd2 = sp.tile([104, 1], F32)
            nc.scalar.activation(sd2[:], var2[:], AF.Sqrt, bias=epsc[:, 0:1], scale=1.0)
            rsd2 = sp.tile([104, 1], F32)
            nc.vector.reciprocal(rsd2[:], sd2[:])
            s2 = sp.tile([104, 1], F32)
            nc.vector.tensor_tensor(s2[:], g2[:], rsd2[:], op=OP.mult)
            ms2 = sp.tile([104, 1], F32)
            nc.vector.tensor_tensor(ms2[:], mean2[:], s2[:], op=OP.mult)
            t2 = sp.tile([104, 1], F32)
            nc.vector.tensor_tensor(t2[:], be2[:], ms2[:], op=OP.subtract)

            # ================= Phase C =================
            with (
                tc.tile_pool(name="psScore", bufs=1, space="PSUM") as psS,
                tc.tile_pool(name="psT", bufs=1, space="PSUM") as psT,
                tc.tile_pool(name="psOut", bufs=1, space="PSUM") as psO,
                tc.tile_pool(name="knr", bufs=3) as knr,
                tc.tile_pool(name="smx", bufs=2) as smx,
            ):
                # h2' slices (sigmoid2 + gate mul), stream into ring
                h2ring = []
                nco = 0
                while nco < RP:
                    w_sl = min(C_SL, RP - nco)
                    sl = slice(nco, nco + w_sl)
                    p2 = wp_pool.tile([104, C_SL], F16, name="p2", tag="actout")
                    nc.scalar.activation(p2[:, 0:w_sl], x2s[:, sl], AF.Sigmoid,
                                         bias=t2[:, 0:1], scale=s2[:, 0:1])
                    if alpha2_nz:
                        nc.vector.tensor_scalar(p2[:, 0:w_sl], p2[:, 0:w_sl],
                                                am2[:, 0:1], am2[:, 1:2],
                                                OP.mult, OP.add)
                    if b2_nz:
                        nc.vector.tensor_scalar(x2s[:, sl], x2s[:, sl],
                                                b2c[:, 0:1], None, OP.add)
                    h2 = wp_pool.tile([104, C_SL], F16, name="h2", tag="gpout", bufs=3)
                    nc.vector.tensor_tensor(h2[:, 0:w_sl], x2s[:, sl], p2[:, 0:w_sl],
                                            op=OP.mult)
                    h2ring.append((nco, w_sl, h2))
                    nco += w_sl

                def h2_slice(col, width):
                    for base, w_sl, t in h2ring:
                        if base <= col and col + width <= base + w_sl:
                            return t[:, col - base:col - base + width]
                    raise AssertionError("h2 slice spans tiles")

                score_ps = [psS.tile([128, 200], F32, name=f"score{g}")
                            for g in range(2)]
                # mm3: paired scores (s, s+2) via dual-column moving operand
                for pgrp in range(S // 4):
                    for sl4 in range(2):
                        cbase = pgrp * CH + sl4 * BSH
                        for g in range(2):
                            st = h2_slice(cbase + g * 128, 128)
                            s0 = pgrp * 4 + sl4
                            nc.tensor.matmul(
                                score_ps[g][:, s0:s0 + 3:2], st, wp2c[:],
                                start=True, stop=True)

                outp = psO.tile([64, BSH], F32)
                outs = smx.tile([64, BSH], F32, name="outs", bufs=1)
                for g in range(2):
                    # softmax over s for 128 batches
                    nmx = smx.tile([128, 1], F32, name="nmx")
                    nc.vector.tensor_reduce(nmx[:], score_ps[g][:], op=OP.max,
                                            axis=mybir.AxisListType.X, negate=True)
                    ex = smx.tile([128, 200], F32, name="ex")
                    se = smx.tile([128, 1], F32, name="se")
                    nc.scalar.activation(ex[:], score_ps[g][:], AF.Exp,
                                         bias=nmx[:, 0:1], scale=1.0,
                                         accum_out=se[:, 0:1])
                    rse = smx.tile([128, 1], F32, name="rse")
                    nc.vector.reciprocal(rse[:], se[:])
                    wgt = smx.tile([128, 200], F16, name="wgt")
                    nc.vector.tensor_scalar(wgt[:], ex[:], rse[:, 0:1], None, OP.mult)
                    # transpose w -> [s, b]
                    wta_p = psT.tile([128, 128], F16, name="wta_p")
                    nc.tensor.transpose(wta_p[:], wgt[:, 0:128], iden[:])
                    wtb_p = psT.tile([72, 128], F16, name="wtb_p")
                    nc.tensor.transpose(wtb_p[:], wgt[:, 128:200], iden[:])
                    wta = smx.tile([128, 128], F16, name="wta")
                    nc.scalar.copy(wta[:], wta_p[:])
                    wtb = smx.tile([72, 128], F16, name="wtb")
                    nc.scalar.copy(wtb[:], wtb_p[:])
                    # einsum per batch
                    for bb in range(0, 128, KNB):
                        kt1 = knr.tile([128, KNB * 64], F16, name="kt1")
                        gb = g * 128 + bb
                        nc.sync.dma_start(kt1[:], kn1_d[:, gb * 64:(gb + KNB) * 64])
                        kt2 = knr.tile([72, KNB * 64], F16, name="kt2")
                        nc.sync.dma_start(kt2[:], kn2_d[:, gb * 64:(gb + KNB) * 64])
                        for bi in range(KNB):
                            bcol = g * 128 + bb + bi
                            nc.tensor.matmul(
                                outp[:, bcol:bcol + 1],
                                kt1[:, bi * 64:(bi + 1) * 64],
                                wta[:, bb + bi:bb + bi + 1],
                                start=True, stop=False)
                            nc.tensor.matmul(
                                outp[:, bcol:bcol + 1],
                                kt2[:, bi * 64:(bi + 1) * 64],
                                wtb[:, bb + bi:bb + bi + 1],
                                start=False, stop=True)
                    nc.scalar.copy(outs[:, g * 128:(g + 1) * 128],
                                   outp[:, g * 128:(g + 1) * 128])
                nc.sync.dma_start(out_d[:, :], outs[:])

    nc.compile()
    return nc


def _prep_inputs(query, keys, W1, b1, gamma1, beta1, alpha1,
                 W2, b2, gamma2, beta2, alpha2, Wp, bp):
    f32 = np.float32
    query = np.asarray(query, f32)
    keys = np.asarray(keys, f32)
    W1 = np.asarray(W1, f32); b1 = np.asarray(b1, f32)
    W2 = np.asarray(W2, f32); b2 = np.asarray(b2, f32)
    Wp = np.asarray(Wp, f32)

    W1a, W1b, W1c, W1d = W1[0:64], W1[64:128], W1[128:192], W1[192:256]
    w1f = np.concatenate([W1b - W1c, W1d], axis=0).astype(np.float16)  # [128, 80]

    q2 = query[:, 0, :]                                  # [B, 64]
    # global mean of xb (exact, fp32)
    mk = keys.reshape(-1, E).mean(0)                     # [64]
    mqk = (keys * query).reshape(-1, E).mean(0)          # [64]
    mu_u = (q2 @ (W1a + W1c) + b1).mean(0)               # [80]
    mean1 = ((W1b - W1c).T @ mk + W1d.T @ mqk + mu_u).astype(f32)

    w2p = np.zeros((H1, 64), np.float16)
    w2p[:, 0:H2] = W2.astype(np.float16)

    wp104 = np.zeros((104, 1), np.float16)
    wp104[0:H2, 0] = Wp[:, 0].astype(np.float16)
    wp104[64:104, 0] = Wp[:, 0].astype(np.float16)
    wp2c = np.zeros((104, 2), np.float16)
    wp2c[0:H2, 0] = Wp[:, 0].astype(np.float16)
    wp2c[64:104, 1] = Wp[:, 0].astype(np.float16)

    def pad104(v, fill):
        out = np.full((104, 1), fill, f32)
        out[0:H2, 0] = v
        out[64:104, 0] = v
        return out

    g2c = pad104(np.asarray(gamma2, f32), 1.0)
    be2c = pad104(np.asarray(beta2, f32), 0.0)
    b2c = pad104(b2, 0.0)
    am2 = np.concatenate([pad104(1.0 - np.asarray(alpha2, f32), 1.0),
                          pad104(np.asarray(alpha2, f32), 0.0)], axis=1)
    am1 = np.stack([1.0 - np.asarray(alpha1, f32), np.asarray(alpha1, f32)],
                   axis=1).astype(f32)

    iden = np.eye(128, dtype=np.float16)

    in_maps = []
    for m in range(M):
        bm = slice(m * BSH, (m + 1) * BSH)
        k_sh = keys[bm]                                  # [256, 200, 64]
        q_sh = q2[bm]                                    # [256, 64]
        kT = np.ascontiguousarray(k_sh.transpose(2, 1, 0).reshape(E, R))
        qkT = np.ascontiguousarray(
            (k_sh * q_sh[:, None, :]).transpose(2, 1, 0).reshape(E, R))
        mov = np.concatenate([kT, qkT], axis=0).astype(np.float16)
        u = np.ascontiguousarray((q_sh @ (W1a + W1c) + b1).T).astype(f32)
        ks = k_sh.transpose(1, 0, 2)                     # [200, 256, 64]
        kn1 = np.ascontiguousarray(ks[0:128].reshape(128, BSH * 64)).astype(np.float16)
        kn2 = np.ascontiguousarray(ks[128:200].reshape(72, BSH * 64)).astype(np.float16)
        in_maps.append(dict(
            mov=mov, w1f=w1f, u=u, mean1=mean1.reshape(H1, 1),
            g1=np.asarray(gamma1, f32).reshape(H1, 1),
            be1=np.asarray(beta1, f32).reshape(H1, 1),
            am1=am1, w2p=w2p, g2=g2c, be2=be2c, am2=am2, b2c=b2c,
            wp=wp104, wp2c=wp2c, kn1=kn1, kn2=kn2, iden=iden,
        ))
    flags = (bool(np.any(np.asarray(alpha1))), bool(np.any(np.asarray(alpha2))),
             bool(np.any(np.asarray(b2))))
    return in_maps, flags


def kernel(**inputs):
    in_maps, flags = _prep_inputs(**inputs)
    if flags not in _CACHE:
        _CACHE[flags] = _build(*flags)
    nc = _CACHE[flags]
    res = run_bass_kernel_spmd(nc, in_maps, core_ids=list(range(M)))
    outs = [res.results[m]["out"].T for m in range(M)]   # [256, 64] each
    return np.concatenate(outs, axis=0).astype(np.float32)


if __name__ == "__main__":
    rng = np.random.default_rng(0)
    pass


# revision 9
# speedup vs baseline: 1.1600x; 1.0046x over previous
"""Trainium2 Bass kernel for nn_AttentionSequence (DIN-style attention, 8 cores).

Data-parallel over batch (2048 -> 8 x 256). Per core, rows are processed in
s-major order (row r = s*256 + b) so the per-batch query term enters as a
fixed [80, 256] matrix add.

Math (per core shard, R = 256*200 = 51200 rows):
  xb = W1f^T mov + U           (mm1 on PE; U-add + PSUM evac on DVE; fp16 stash)
  Sxb2 = sum xb^2              (ACT square + accum_out)
  AR1: global sum of Sxb2; mean1 is host-precomputed (linear in inputs)
  p1 = sigmoid(s1*xb + t1)     (ACT), h1 = xb*p1 (GPSIMD)  [alpha1=0 fast path]
  x2 = W2p^T h1                (mm2, pairs packed at partitions 0:64/64:104)
  stats2 via bn_stats/bn_aggr  (DVE), AR2
  p2 = sigmoid(s2*x2+t2), h2 = x2*p2
  score[s,b] via mm3 (stationary h2-slice [40,128], moving Wp) -> PSUM [128b, 200s]
  softmax over s (max-subtract, exp with accum sum)
  out[e,b] = sum_s w[s,b]*keys[b,s,e] via per-batch 2-matmul einsum
"""
import numpy as np

import concourse.bacc as bacc
import concourse.tile as tile
import concourse.mybir as mybir
from concourse.bass_utils import run_bass_kernel_spmd

F16 = mybir.dt.float16
F32 = mybir.dt.float32
AF = mybir.ActivationFunctionType
OP = mybir.AluOpType

M = 8
B, S, E = 2048, 200, 64
H1, H2 = 80, 40
BSH = B // M            # 256 batches per core
R = BSH * S             # 51200 rows per core
CH = 512                # chunk rows (= 2 s-values x 256 batches)
NCH = R // CH           # 100 chunks
NPAIR = NCH // 2        # 50 chunk pairs
RP = R // 2             # 25600 stash cols for packed x2
EPS = 1e-5
NTOT = float(B * S)

SQ_SL = 2048            # ACT square slice
NSQ = R // SQ_SL        # 25
SG_SL = 2048            # sigmoid1/gp-mul slice
NSG = R // SG_SL        # 25
C_SL = 2048             # phase C slice over RP
NC_SL = RP // C_SL      # 12.5 -> handle remainder
KNB = 16                # kn batches per streamed block

_CACHE = {}


def _build(alpha1_nz, alpha2_nz, b2_nz):
    nc = bacc.Bacc()

    mov_d = nc.declare_dram_parameter("mov", [128, R], F16, isOutput=False)
    w1f_d = nc.declare_dram_parameter("w1f", [128, H1], F16, isOutput=False)
    u_d = nc.declare_dram_parameter("u", [H1, BSH], F32, isOutput=False)
    mean1_d = nc.declare_dram_parameter("mean1", [H1, 1], F32, isOutput=False)
    g1_d = nc.declare_dram_parameter("g1", [H1, 1], F32, isOutput=False)
    be1_d = nc.declare_dram_parameter("be1", [H1, 1], F32, isOutput=False)
    am1_d = nc.declare_dram_parameter("am1", [H1, 2], F32, isOutput=False)
    w2p_d = nc.declare_dram_parameter("w2p", [H1, 64], F16, isOutput=False)
    g2_d = nc.declare_dram_parameter("g2", [104, 1], F32, isOutput=False)
    be2_d = nc.declare_dram_parameter("be2", [104, 1], F32, isOutput=False)
    am2_d = nc.declare_dram_parameter("am2", [104, 2], F32, isOutput=False)
    b2c_d = nc.declare_dram_parameter("b2c", [104, 1], F32, isOutput=False)
    wp_d = nc.declare_dram_parameter("wp", [104, 1], F16, isOutput=False)
    wp2c_d = nc.declare_dram_parameter("wp2c", [104, 2], F16, isOutput=False)
    kn1_d = nc.declare_dram_parameter("kn1", [128, BSH * 64], F16, isOutput=False)
    kn2_d = nc.declare_dram_parameter("kn2", [72, BSH * 64], F16, isOutput=False)
    iden_d = nc.declare_dram_parameter("iden", [128, 128], F16, isOutput=False)

    out_d = nc.declare_dram_parameter("out", [64, BSH], F32, isOutput=True)

    ar1a_in = nc.dram_tensor("ar1a_in", [H1, 1], F32)
    ar1a_out = nc.dram_tensor("ar1a_out", [H1, 1], F32, addr_space="Shared")
    ar1_in = nc.dram_tensor("ar1_in", [H1, 1], F32)
    ar1_out = nc.dram_tensor("ar1_out", [H1, 1], F32, addr_space="Shared")
    ar2a_in = nc.dram_tensor("ar2a_in", [104, 2], F32)
    ar2a_out = nc.dram_tensor("ar2a_out", [104, 2], F32, addr_space="Shared")
    ar2_in = nc.dram_tensor("ar2_in", [104, 2], F32)
    ar2_out = nc.dram_tensor("ar2_out", [104, 2], F32, addr_space="Shared")

    with tile.TileContext(nc) as tc:
        with (
            tc.tile_pool(name="const", bufs=1) as cp,
            tc.tile_pool(name="stash", bufs=1) as stp,
            tc.tile_pool(name="work", bufs=2) as wp_pool,
            tc.tile_pool(name="movr", bufs=6) as movr,
            tc.tile_pool(name="stats", bufs=1) as sp,
        ):
            # ---- constants ----
            w1f = cp.tile([128, H1], F16)
            nc.sync.dma_start(w1f[:], w1f_d[:, :])
            u4 = cp.tile([H1, 2 * CH], F32)
            for _r in range(4):
                nc.sync.dma_start(u4[:, _r * BSH:(_r + 1) * BSH], u_d[:, :])
            w2p = cp.tile([H1, 64], F16)
            nc.sync.dma_start(w2p[:], w2p_d[:, :])
            wp2c = cp.tile([104, 2], F16)
            nc.sync.dma_start(wp2c[:], wp2c_d[:, :])
            iden = cp.tile([128, 128], F16)
            nc.sync.dma_start(iden[:], iden_d[:, :])
            mean1 = sp.tile([H1, 1], F32)
            nc.sync.dma_start(mean1[:], mean1_d[:, :])
            g1 = sp.tile([H1, 1], F32)
            nc.sync.dma_start(g1[:], g1_d[:, :])
            be1 = sp.tile([H1, 1], F32)
            nc.sync.dma_start(be1[:], be1_d[:, :])
            g2 = sp.tile([104, 1], F32)
            nc.sync.dma_start(g2[:], g2_d[:, :])
            be2 = sp.tile([104, 1], F32)
            nc.sync.dma_start(be2[:], be2_d[:, :])
            if alpha1_nz:
                am1 = sp.tile([H1, 2], F32)
                nc.sync.dma_start(am1[:], am1_d[:, :])
            if alpha2_nz:
                am2 = sp.tile([104, 2], F32)
                nc.sync.dma_start(am2[:], am2_d[:, :])
            if b2_nz:
                b2c = sp.tile([104, 1], F32)
                nc.sync.dma_start(b2c[:], b2c_d[:, :])

            # ---- big stashes ----
            xb = stp.tile([H1, R], F16)          # layer-1 pre-BN activations
            x2s = stp.tile([104, RP], F16)       # packed layer-2 pre-BN
            sqcols = sp.tile([H1, 32], F32)      # ACT square accum partials
            nc.vector.memset(sqcols[:], 0.0)
            epsc = sp.tile([104, 1], F32)
            nc.vector.memset(epsc[:], EPS)
            bns = sp.tile([104, 6 * NPAIR], F32)  # bn_stats partials

            # ================= Phase A =================
            with tc.tile_pool(name="psA", bufs=3, space="PSUM") as psA:
                for j2 in range(NCH // 2):
                    x1p = psA.tile([H1, 2 * CH], F32, name="x1p")
                    for k2 in range(2):
                        j = j2 * 2 + k2
                        mv = movr.tile([128, CH], F16, name="mv")
                        nc.sync.dma_start(mv[:], mov_d[:, j * CH:(j + 1) * CH])
                        nc.tensor.matmul(x1p[:, k2 * CH:(k2 + 1) * CH], w1f[:],
                                         mv[:], start=True, stop=True)
                    # evac + U add (fp32 psum + fp32 U -> fp16 stash)
                    nc.vector.tensor_tensor(
                        xb[:, j2 * 2 * CH:(j2 + 1) * 2 * CH], x1p[:], u4[:],
                        op=OP.add)
                    if j2 % 2 == 1:
                        k = j2 // 2
                        sqj = wp_pool.tile([H1, SQ_SL], F16, name="sqj", tag="actout")
                        nc.scalar.activation(
                            sqj[:], xb[:, k * SQ_SL:(k + 1) * SQ_SL], AF.Square,
                            accum_out=sqcols[:, k:k + 1])
                        if k == 12:
                            # first-half sum-of-squares: early AllReduce
                            sxa = sp.tile([H1, 1], F32, name="sxa")
                            nc.vector.tensor_reduce(
                                sxa[:], sqcols[:, 0:13],
                                axis=mybir.AxisListType.X, op=OP.add)
                            nc.sync.dma_start(ar1a_in[:, :], sxa[:])
                            nc.gpsimd.collective_compute(
                                "AllReduce", OP.add,
                                replica_groups=[list(range(M))],
                                ins=[ar1a_in[:, :]], outs=[ar1a_out[:, :]])

            # ---- AR1b: second-half sum of squares ----
            sx2g = sp.tile([H1, 1], F32)
            nc.vector.tensor_reduce(sx2g[:], sqcols[:, 13:32],
                                    axis=mybir.AxisListType.X, op=OP.add)
            nc.sync.dma_start(ar1_in[:, :], sx2g[:])
            nc.gpsimd.collective_compute(
                "AllReduce", OP.add, replica_groups=[list(range(M))],
                ins=[ar1_in[:, :]], outs=[ar1_out[:, :]])
            sx2a = sp.tile([H1, 1], F32)
            nc.sync.dma_start(sx2a[:], ar1_out[:, :])
            sx2aa = sp.tile([H1, 1], F32)
            nc.sync.dma_start(sx2aa[:], ar1a_out[:, :])
            nc.vector.tensor_tensor(sx2a[:], sx2a[:], sx2aa[:], op=OP.add)

            # stats1: var = E[x^2] - mean1^2 ; s1 = g1/sqrt(var+eps) ; t1 = be1 - mean1*s1
            ex2 = sp.tile([H1, 1], F32)
            nc.vector.tensor_scalar(ex2[:], sx2a[:], 1.0 / NTOT, None, OP.mult)
            msq = sp.tile([H1, 1], F32)
            nc.vector.tensor_tensor(msq[:], mean1[:], mean1[:], op=OP.mult)
            var1 = sp.tile([H1, 1], F32)
            nc.vector.tensor_tensor(var1[:], ex2[:], msq[:], op=OP.subtract)
            sd1 = sp.tile([H1, 1], F32)
            nc.scalar.activation(sd1[:], var1[:], AF.Sqrt, bias=epsc[0:H1, 0:1], scale=1.0)
            rsd1 = sp.tile([H1, 1], F32)
            nc.vector.reciprocal(rsd1[:], sd1[:])
            s1 = sp.tile([H1, 1], F32)
            nc.vector.tensor_tensor(s1[:], g1[:], rsd1[:], op=OP.mult)
            ms1 = sp.tile([H1, 1], F32)
            nc.vector.tensor_tensor(ms1[:], mean1[:], s1[:], op=OP.mult)
            t1 = sp.tile([H1, 1], F32)
            nc.vector.tensor_tensor(t1[:], be1[:], ms1[:], op=OP.subtract)

            # ================= Phase B =================
            zc = sp.tile([104, 1], F32)
            nc.vector.memset(zc[:], 0.0)
            with tc.tile_pool(name="psB", bufs=3, space="PSUM") as psB:
                for blk in range(NSG):          # 25 blocks of 2048 rows = 2 pairs
                    sl = slice(blk * SG_SL, (blk + 1) * SG_SL)
                    p1 = wp_pool.tile([H1, SG_SL], F16, name="p1", tag="actout")
                    nc.scalar.activation(p1[:], xb[:, sl], AF.Sigmoid,
                                         bias=t1[:, 0:1], scale=s1[:, 0:1])
                    if alpha1_nz:
                        nc.vector.tensor_scalar(p1[:], p1[:], am1[:, 0:1],
                                                am1[:, 1:2], OP.mult, OP.add)
                    h1 = wp_pool.tile([H1, SG_SL], F16, name="h1", tag="gpout", bufs=3)
                    SPL = 1536
                    nc.vector.tensor_tensor(
                        h1[:, 0:SPL], xb[:, blk * SG_SL:blk * SG_SL + SPL],
                        p1[:, 0:SPL], op=OP.mult)
                    nc.gpsimd.tensor_tensor(
                        h1[:, SPL:SG_SL], xb[:, blk * SG_SL + SPL:(blk + 1) * SG_SL],
                        p1[:, SPL:SG_SL], op=OP.mult)
                    for pp in range(2):         # 2 chunk-pairs per block
                        p = blk * 2 + pp
                        x2p = psB.tile([104, CH], F32, name="x2p")
                        c0 = pp * 2 * CH
                        nc.tensor.matmul(x2p[0:64, :], w2p[:],
                                         h1[:, c0:c0 + CH], start=True, stop=True)
                        nc.tensor.matmul(x2p[64:104, :], w2p[:, 0:H2],
                                         h1[:, c0 + CH:c0 + 2 * CH], start=True,
                                         stop=True, tile_position=(0, 64))
                        if pp == 0:
                            nc.scalar.copy(x2s[:, p * CH:(p + 1) * CH], x2p[:])
                        else:
                            nc.vector.tensor_tensor(
                                x2s[:, p * CH:(p + 1) * CH], x2p[:],
                                zc[:].broadcast_to([104, CH]), op=OP.add)
                        nc.vector.bn_stats(bns[:, p * 6:(p + 1) * 6],
                                           x2s[:, p * CH:(p + 1) * CH])
                        if p == 24:
                            bnaa = sp.tile([104, 2], F32, name="bnaa")
                            nc.vector.bn_aggr(bnaa[:], bns[:, 0:150])
                            sna = sp.tile([104, 2], F32, name="sna")
                            nc.vector.tensor_scalar(sna[:, 0:1], bnaa[:, 0:1],
                                                    float(R // 2), None, OP.mult)
                            mqa = sp.tile([104, 1], F32, name="mqa")
                            nc.vector.tensor_tensor(mqa[:], bnaa[:, 0:1],
                                                    bnaa[:, 0:1], op=OP.mult)
                            nc.vector.tensor_tensor(mqa[:], bnaa[:, 1:2], mqa[:],
                                                    op=OP.add)
                            nc.vector.tensor_scalar(sna[:, 1:2], mqa[:],
                                                    float(R // 2), None, OP.mult)
                            nc.sync.dma_start(ar2a_in[:, :], sna[:])
                            nc.gpsimd.collective_compute(
                                "AllReduce", OP.add,
                                replica_groups=[list(range(M))],
                                ins=[ar2a_in[:, :]], outs=[ar2a_out[:, :]])

            # ---- AR2b: layer-2 stats (second half) ----
            bna = sp.tile([104, 2], F32)
            nc.vector.bn_aggr(bna[:], bns[:, 150:300])
            # convert (mean, var) -> (sum, sumsq) scaled by local count R
            s2s = sp.tile([104, 2], F32)
            nc.vector.tensor_scalar(s2s[:, 0:1], bna[:, 0:1], float(R // 2), None, OP.mult)
            m2sq = sp.tile([104, 1], F32)
            nc.vector.tensor_tensor(m2sq[:], bna[:, 0:1], bna[:, 0:1], op=OP.mult)
            nc.vector.tensor_tensor(m2sq[:], bna[:, 1:2], m2sq[:], op=OP.add)
            nc.vector.tensor_scalar(s2s[:, 1:2], m2sq[:], float(R // 2), None, OP.mult)
            nc.sync.dma_start(ar2_in[:, :], s2s[:])
            nc.gpsimd.collective_compute(
                "AllReduce", OP.add, replica_groups=[list(range(M))],
                ins=[ar2_in[:, :]], outs=[ar2_out[:, :]])
            s2a = sp.tile([104, 2], F32)
            nc.sync.dma_start(s2a[:], ar2_out[:, :])
            s2aa = sp.tile([104, 2], F32)
            nc.sync.dma_start(s2aa[:], ar2a_out[:, :])
            nc.vector.tensor_tensor(s2a[:], s2a[:], s2aa[:], op=OP.add)

            mean2 = sp.tile([104, 1], F32)
            nc.vector.tensor_scalar(mean2[:], s2a[:, 0:1], 1.0 / NTOT, None, OP.mult)
            ex22 = sp.tile([104, 1], F32)
            nc.vector.tensor_scalar(ex22[:], s2a[:, 1:2], 1.0 / NTOT, None, OP.mult)
            msq2 = sp.tile([104, 1], F32)
            nc.vector.tensor_tensor(msq2[:], mean2[:], mean2[:], op=OP.mult)
            var2 = sp.tile([104, 1], F32)
            nc.vector.tensor_tensor(var2[:], ex22[:], msq2[:], op=OP.subtract)
            sd2 = sp.tile([104, 1], F32)
            nc.scalar.activation(sd2[:], var2[:], AF.Sqrt, bias=epsc[:, 0:1], scale=1.0)
            rsd2 = sp.tile([104, 1], F32)
            nc.vector.reciprocal(rsd2[:], sd2[:])
            s2 = sp.tile([104, 1], F32)
            nc.vector.tensor_tensor(s2[:], g2[:], rsd2[:], op=OP.mult)
            ms2 = sp.tile([104, 1], F32)
            nc.vector.tensor_tensor(ms2[:], mean2[:], s2[:], op=OP.mult)
            t2 = sp.tile([104, 1], F32)
            nc.vector.tensor_tensor(t2[:], be2[:], ms2[:], op=OP.subtract)

            # ================= Phase C =================
            with (
                tc.tile_pool(name="psScore", bufs=1, space="PSUM") as psS,
                tc.tile_pool(name="psT", bufs=1, space="PSUM") as psT,
                tc.tile_pool(name="psOut", bufs=1, space="PSUM") as psO,
                tc.tile_pool(name="knr", bufs=3) as knr,
                tc.tile_pool(name="smx", bufs=2) as smx,
            ):
                # h2' slices (sigmoid2 + gate mul), stream into ring
                h2ring = []
                nco = 0
                while nco < RP:
                    w_sl = min(C_SL, RP - nco)
                    sl = slice(nco, nco + w_sl)
                    p2 = wp_pool.tile([104, C_SL], F16, name="p2", tag="actout")
                    nc.scalar.activation(p2[:, 0:w_sl], x2s[:, sl], AF.Sigmoid,
                                         bias=t2[:, 0:1], scale=s2[:, 0:1])
                    if alpha2_nz:
                        nc.vector.tensor_scalar(p2[:, 0:w_sl], p2[:, 0:w_sl],
                                                am2[:, 0:1], am2[:, 1:2],
                                                OP.mult, OP.add)
                    if b2_nz:
                        nc.vector.tensor_scalar(x2s[:, sl], x2s[:, sl],
                                                b2c[:, 0:1], None, OP.add)
                    h2 = wp_pool.tile([104, C_SL], F16, name="h2", tag="gpout", bufs=3)
                    nc.vector.tensor_tensor(h2[:, 0:w_sl], x2s[:, sl], p2[:, 0:w_sl],
                                            op=OP.mult)
                    h2ring.append((nco, w_sl, h2))
                    nco += w_sl

                def h2_slice(col, width):
                    for base, w_sl, t in h2ring:
                        if base <= col and col + width <= base + w_sl:
                            return t[:, col - base:col - base + width]
                    raise AssertionError("h2 slice spans tiles")

                score_ps = [psS.tile([128, 200], F32, name=f"score{g}")
                            for g in range(2)]
                # mm3: paired scores (s, s+2) via dual-column moving operand
                for pgrp in range(S // 4):
                    for sl4 in range(2):
                        cbase = pgrp * CH + sl4 * BSH
                        for g in range(2):
                            st = h2_slice(cbase + g * 128, 128)
                            s0 = pgrp * 4 + sl4
                            nc.tensor.matmul(
                                score_ps[g][:, s0:s0 + 3:2], st, wp2c[:],
                                start=True, stop=True)

                outp = psO.tile([64, BSH], F32)
                outs = smx.tile([64, BSH], F32, name="outs", bufs=1)
                for g in range(2):
                    # softmax over s for 128 batches
                    nmx = smx.tile([128, 1], F32, name="nmx")
                    nc.vector.tensor_reduce(nmx[:], score_ps[g][:], op=OP.max,
                                            axis=mybir.AxisListType.X, negate=True)
                    ex = smx.tile([128, 200], F32, name="ex")
                    se = smx.tile([128, 1], F32, name="se")
                    nc.scalar.activation(ex[:], score_ps[g][:], AF.Exp,
                                         bias=nmx[:, 0:1], scale=1.0,
                                         accum_out=se[:, 0:1])
                    rse = smx.tile([128, 1], F32, name="rse")
                    nc.vector.reciprocal(rse[:], se[:])
                    wgt = smx.tile([128, 200], F16, name="wgt")
                    nc.vector.tensor_scalar(wgt[:], ex[:], rse[:, 0:1], None, OP.mult)
                    # transpose w -> [s, b]
                    wta_p = psT.tile([128, 128], F16, name="wta_p")
                    nc.tensor.transpose(wta_p[:], wgt[:, 0:128], iden[:])
                    wtb_p = psT.tile([72, 128], F16, name="wtb_p")
                    nc.tensor.transpose(wtb_p[:], wgt[:, 128:200], iden[:])
                    wta = smx.tile([128, 128], F16, name="wta")
                    nc.scalar.copy(wta[:], wta_p[:])
                    wtb = smx.tile([72, 128], F16, name="wtb")
                    nc.scalar.copy(wtb[:], wtb_p[:])
                    # einsum per batch
                    for bb in range(0, 128, KNB):
                        kt1 = knr.tile([128, KNB * 64], F16, name="kt1")
                        gb = g * 128 + bb
                        nc.sync.dma_start(kt1[:], kn1_d[:, gb * 64:(gb + KNB) * 64])
                        kt2 = knr.tile([72, KNB * 64], F16, name="kt2")
                        nc.sync.dma_start(kt2[:], kn2_d[:, gb * 64:(gb + KNB) * 64])
                        for bi in range(KNB):
                            bcol = g * 128 + bb + bi
                            nc.tensor.matmul(
                                outp[:, bcol:bcol + 1],
                                kt1[:, bi * 64:(bi + 1) * 64],
                                wta[:, bb + bi:bb + bi + 1],
                                start=True, stop=False)
                            nc.tensor.matmul(
                                outp[:, bcol:bcol + 1],
                                kt2[:, bi * 64:(bi + 1) * 64],
                                wtb[:, bb + bi:bb + bi + 1],
                                start=False, stop=True)
                    nc.scalar.copy(outs[:, g * 128:(g + 1) * 128],
                                   outp[:, g * 128:(g + 1) * 128])
                nc.sync.dma_start(out_d[:, :], outs[:])

    nc.compile()
    return nc


def _prep_inputs(query, keys, W1, b1, gamma1, beta1, alpha1,
                 W2, b2, gamma2, beta2, alpha2, Wp, bp):
    f32 = np.float32
    query = np.asarray(query, f32)
    keys = np.asarray(keys, f32)
    W1 = np.asarray(W1, f32); b1 = np.asarray(b1, f32)
    W2 = np.asarray(W2, f32); b2 = np.asarray(b2, f32)
    Wp = np.asarray(Wp, f32)

    W1a, W1b, W1c, W1d = W1[0:64], W1[64:128], W1[128:192], W1[192:256]
    w1f = np.concatenate([W1b - W1c, W1d], axis=0).astype(np.float16)  # [128, 80]

    q2 = query[:, 0, :]                                  # [B, 64]
    # global mean of xb (exact, fp32)
    mk = keys.reshape(-1, E).mean(0)                     # [64]
    mqk = (keys * query).reshape(-1, E).mean(0)          # [64]
    mu_u = (q2 @ (W1a + W1c) + b1).mean(0)               # [80]
    mean1 = ((W1b - W1c).T @ mk + W1d.T @ mqk + mu_u).astype(f32)

    w2p = np.zeros((H1, 64), np.float16)
    w2p[:, 0:H2] = W2.astype(np.float16)

    wp104 = np.zeros((104, 1), np.float16)
    wp104[0:H2, 0] = Wp[:, 0].astype(np.float16)
    wp104[64:104, 0] = Wp[:, 0].astype(np.float16)
    wp2c = np.zeros((104, 2), np.float16)
    wp2c[0:H2, 0] = Wp[:, 0].astype(np.float16)
    wp2c[64:104, 1] = Wp[:, 0].astype(np.float16)

    def pad104(v, fill):
        out = np.full((104, 1), fill, f32)
        out[0:H2, 0] = v
        out[64:104, 0] = v
        return out

    g2c = pad104(np.asarray(gamma2, f32), 1.0)
    be2c = pad104(np.asarray(beta2, f32), 0.0)
    b2c = pad104(b2, 0.0)
    am2 = np.concatenate([pad104(1.0 - np.asarray(alpha2, f32), 1.0),
                          pad104(np.asarray(alpha2, f32), 0.0)], axis=1)
    am1 = np.stack([1.0 - np.asarray(alpha1, f32), np.asarray(alpha1, f32)],
                   axis=1).astype(f32)

    iden = np.eye(128, dtype=np.float16)

    in_maps = []
    for m in range(M):
        bm = slice(m * BSH, (m + 1) * BSH)
        k_sh = keys[bm]                                  # [256, 200, 64]
        q_sh = q2[bm]                                    # [256, 64]
        kT = np.ascontiguousarray(k_sh.transpose(2, 1, 0).reshape(E, R))
        qkT = np.ascontiguousarray(
            (k_sh * q_sh[:, None, :]).transpose(2, 1, 0).reshape(E, R))
        mov = np.concatenate([kT, qkT], axis=0).astype(np.float16)
        u = np.ascontiguousarray((q_sh @ (W1a + W1c) + b1).T).astype(f32)
        ks = k_sh.transpose(1, 0, 2)                     # [200, 256, 64]
        kn1 = np.ascontiguousarray(ks[0:128].reshape(128, BSH * 64)).astype(np.float16)
        kn2 = np.ascontiguousarray(ks[128:200].reshape(72, BSH * 64)).astype(np.float16)
        in_maps.append(dict(
            mov=mov, w1f=w1f, u=u, mean1=mean1.reshape(H1, 1),
            g1=np.asarray(gamma1, f32).reshape(H1, 1),
            be1=np.asarray(beta1, f32).reshape(H1, 1),
            am1=am1, w2p=w2p, g2=g2c, be2=be2c, am2=am2, b2c=b2c,
            wp=wp104, wp2c=wp2c, kn1=kn1, kn2=kn2, iden=iden,
        ))
    flags = (bool(np.any(np.asarray(alpha1))), bool(np.any(np.asarray(alpha2))),
             bool(np.any(np.asarray(b2))))
    return in_maps, flags


def kernel(**inputs):
    in_maps, flags = _prep_inputs(**inputs)
    if flags not in _CACHE:
        _CACHE[flags] = _build(*flags)
    nc = _CACHE[flags]
    res = run_bass_kernel_spmd(nc, in_maps, core_ids=list(range(M)))
    outs = [res.results[m]["out"].T for m in range(M)]   # [256, 64] each
    return np.concatenate(outs, axis=0).astype(np.float32)


if __name__ == "__main__":
    rng = np.random.default_rng(0)
    pass


# revision 12
# speedup vs baseline: 1.1891x; 1.0251x over previous
"""Trainium2 Bass kernel for nn_AttentionSequence (DIN-style attention, 8 cores).

Data-parallel over batch (2048 -> 8 x 256). Per core, rows are processed in
s-major order (row r = s*256 + b) so the per-batch query term enters as a
fixed [80, 256] matrix add.

Math (per core shard, R = 256*200 = 51200 rows):
  xb = W1f^T mov + U           (mm1 on PE; U-add + PSUM evac on DVE; fp16 stash)
  Sxb2 = sum xb^2              (ACT square + accum_out)
  AR1: global sum of Sxb2; mean1 is host-precomputed (linear in inputs)
  p1 = sigmoid(s1*xb + t1)     (ACT), h1 = xb*p1 (GPSIMD)  [alpha1=0 fast path]
  x2 = W2p^T h1                (mm2, pairs packed at partitions 0:64/64:104)
  stats2 via bn_stats/bn_aggr  (DVE), AR2
  p2 = sigmoid(s2*x2+t2), h2 = x2*p2
  score[s,b] via mm3 (stationary h2-slice [40,128], moving Wp) -> PSUM [128b, 200s]
  softmax over s (max-subtract, exp with accum sum)
  out[e,b] = sum_s w[s,b]*keys[b,s,e] via per-batch 2-matmul einsum
"""
import numpy as np

import concourse.bacc as bacc
import concourse.tile as tile
import concourse.mybir as mybir
from concourse.bass_utils import run_bass_kernel_spmd

F16 = mybir.dt.float16
F32 = mybir.dt.float32
AF = mybir.ActivationFunctionType
OP = mybir.AluOpType

M = 8
B, S, E = 2048, 200, 64
H1, H2 = 80, 40
BSH = B // M            # 256 batches per core
R = BSH * S             # 51200 rows per core
CH = 512                # chunk rows (= 2 s-values x 256 batches)
NCH = R // CH           # 100 chunks
NPAIR = NCH // 2        # 50 chunk pairs
RP = R // 2             # 25600 stash cols for packed x2
EPS = 1e-5
NTOT = float(B * S)

SQ_SL = 2048            # ACT square slice
NSQ = R // SQ_SL        # 25
SG_SL = 2048            # sigmoid1/gp-mul slice
NSG = R // SG_SL        # 25
C_SL = 2048             # phase C slice over RP
NC_SL = RP // C_SL      # 12.5 -> handle remainder
KNB = 16                # kn batches per streamed block

_CACHE = {}


def _build(alpha1_nz, alpha2_nz, b2_nz):
    nc = bacc.Bacc()

    mov_d = nc.declare_dram_parameter("mov", [128, R], F16, isOutput=False)
    w1f_d = nc.declare_dram_parameter("w1f", [128, H1], F16, isOutput=False)
    u_d = nc.declare_dram_parameter("u", [H1, BSH], F32, isOutput=False)
    mean1_d = nc.declare_dram_parameter("mean1", [H1, 1], F32, isOutput=False)
    g1_d = nc.declare_dram_parameter("g1", [H1, 1], F32, isOutput=False)
    be1_d = nc.declare_dram_parameter("be1", [H1, 1], F32, isOutput=False)
    am1_d = nc.declare_dram_parameter("am1", [H1, 2], F32, isOutput=False)
    w2p_d = nc.declare_dram_parameter("w2p", [H1, 64], F16, isOutput=False)
    g2_d = nc.declare_dram_parameter("g2", [104, 1], F32, isOutput=False)
    be2_d = nc.declare_dram_parameter("be2", [104, 1], F32, isOutput=False)
    am2_d = nc.declare_dram_parameter("am2", [104, 2], F32, isOutput=False)
    b2c_d = nc.declare_dram_parameter("b2c", [104, 1], F32, isOutput=False)
    wp_d = nc.declare_dram_parameter("wp", [104, 1], F16, isOutput=False)
    wp2c_d = nc.declare_dram_parameter("wp2c", [104, 2], F16, isOutput=False)
    kn1_d = nc.declare_dram_parameter("kn1", [128, BSH * 64], F16, isOutput=False)
    kn2_d = nc.declare_dram_parameter("kn2", [72, BSH * 64], F16, isOutput=False)
    iden_d = nc.declare_dram_parameter("iden", [128, 128], F16, isOutput=False)

    out_d = nc.declare_dram_parameter("out", [64, BSH], F32, isOutput=True)

    ar1a_in = nc.dram_tensor("ar1a_in", [H1, 1], F32)
    ar1a_out = nc.dram_tensor("ar1a_out", [H1, 1], F32, addr_space="Shared")
    ar1_in = nc.dram_tensor("ar1_in", [H1, 1], F32)
    ar1_out = nc.dram_tensor("ar1_out", [H1, 1], F32, addr_space="Shared")
    ar2a_in = nc.dram_tensor("ar2a_in", [104, 2], F32)
    ar2a_out = nc.dram_tensor("ar2a_out", [104, 2], F32, addr_space="Shared")
    ar2_in = nc.dram_tensor("ar2_in", [104, 2], F32)
    ar2_out = nc.dram_tensor("ar2_out", [104, 2], F32, addr_space="Shared")

    with tile.TileContext(nc) as tc:
        with (
            tc.tile_pool(name="const", bufs=1) as cp,
            tc.tile_pool(name="stash", bufs=1) as stp,
            tc.tile_pool(name="work", bufs=2) as wp_pool,
            tc.tile_pool(name="movr", bufs=6) as movr,
            tc.tile_pool(name="stats", bufs=1) as sp,
        ):
            # ---- constants ----
            w1f = cp.tile([128, H1], F16)
            nc.sync.dma_start(w1f[:], w1f_d[:, :])
            u4 = cp.tile([H1, 2 * CH], F32)
            for _r in range(4):
                nc.sync.dma_start(u4[:, _r * BSH:(_r + 1) * BSH], u_d[:, :])
            w2p = cp.tile([H1, 64], F16)
            nc.sync.dma_start(w2p[:], w2p_d[:, :])
            wp2c = cp.tile([104, 2], F16)
            nc.sync.dma_start(wp2c[:], wp2c_d[:, :])
            iden = cp.tile([128, 128], F16)
            nc.sync.dma_start(iden[:], iden_d[:, :])
            mean1 = sp.tile([H1, 1], F32)
            nc.sync.dma_start(mean1[:], mean1_d[:, :])
            g1 = sp.tile([H1, 1], F32)
            nc.sync.dma_start(g1[:], g1_d[:, :])
            be1 = sp.tile([H1, 1], F32)
            nc.sync.dma_start(be1[:], be1_d[:, :])
            g2 = sp.tile([104, 1], F32)
            nc.sync.dma_start(g2[:], g2_d[:, :])
            be2 = sp.tile([104, 1], F32)
            nc.sync.dma_start(be2[:], be2_d[:, :])
            if alpha1_nz:
                am1 = sp.tile([H1, 2], F32)
                nc.sync.dma_start(am1[:], am1_d[:, :])
            if alpha2_nz:
                am2 = sp.tile([104, 2], F32)
                nc.sync.dma_start(am2[:], am2_d[:, :])
            if b2_nz:
                b2c = sp.tile([104, 1], F32)
                nc.sync.dma_start(b2c[:], b2c_d[:, :])

            # ---- big stashes ----
            xb = stp.tile([H1, R], F16)          # layer-1 pre-BN activations
            x2s = stp.tile([104, RP], F16)       # packed layer-2 pre-BN
            sqcols = sp.tile([H1, 32], F32)      # ACT square accum partials
            nc.vector.memset(sqcols[:], 0.0)
            epsc = sp.tile([104, 1], F32)
            nc.vector.memset(epsc[:], EPS)
            bns = sp.tile([104, 6 * NPAIR], F32)  # bn_stats partials

            # ================= Phase A =================
            with tc.tile_pool(name="psA", bufs=3, space="PSUM") as psA:
                for j2 in range(NCH // 2):
                    x1p = psA.tile([H1, 2 * CH], F32, name="x1p")
                    for k2 in range(2):
                        j = j2 * 2 + k2
                        mv = movr.tile([128, CH], F16, name="mv")
                        nc.sync.dma_start(mv[:], mov_d[:, j * CH:(j + 1) * CH])
                        nc.tensor.matmul(x1p[:, k2 * CH:(k2 + 1) * CH], w1f[:],
                                         mv[:], start=True, stop=True)
                    # evac + U add (fp32 psum + fp32 U -> fp16 stash)
                    nc.vector.tensor_tensor(
                        xb[:, j2 * 2 * CH:(j2 + 1) * 2 * CH], x1p[:], u4[:],
                        op=OP.add)
                    if j2 % 2 == 1:
                        k = j2 // 2
                        sqj = wp_pool.tile([H1, SQ_SL], F16, name="sqj", tag="actout")
                        nc.scalar.activation(
                            sqj[:], xb[:, k * SQ_SL:(k + 1) * SQ_SL], AF.Square,
                            accum_out=sqcols[:, k:k + 1])
                        if k == 22:
                            # first-half sum-of-squares: early AllReduce
                            sxa = sp.tile([H1, 1], F32, name="sxa")
                            nc.vector.tensor_reduce(
                                sxa[:], sqcols[:, 0:23],
                                axis=mybir.AxisListType.X, op=OP.add)
                            nc.sync.dma_start(ar1a_in[:, :], sxa[:])
                            nc.gpsimd.collective_compute(
                                "AllReduce", OP.add,
                                replica_groups=[list(range(M))],
                                ins=[ar1a_in[:, :]], outs=[ar1a_out[:, :]])

            # ---- AR1b: second-half sum of squares ----
            sx2g = sp.tile([H1, 1], F32)
            nc.vector.tensor_reduce(sx2g[:], sqcols[:, 23:32],
                                    axis=mybir.AxisListType.X, op=OP.add)
            nc.sync.dma_start(ar1_in[:, :], sx2g[:])
            nc.gpsimd.collective_compute(
                "AllReduce", OP.add, replica_groups=[list(range(M))],
                ins=[ar1_in[:, :]], outs=[ar1_out[:, :]])
            sx2a = sp.tile([H1, 1], F32)
            nc.sync.dma_start(sx2a[:], ar1_out[:, :])
            sx2aa = sp.tile([H1, 1], F32)
            nc.sync.dma_start(sx2aa[:], ar1a_out[:, :])
            nc.vector.tensor_tensor(sx2a[:], sx2a[:], sx2aa[:], op=OP.add)

            # stats1: var = E[x^2] - mean1^2 ; s1 = g1/sqrt(var+eps) ; t1 = be1 - mean1*s1
            ex2 = sp.tile([H1, 1], F32)
            nc.vector.tensor_scalar(ex2[:], sx2a[:], 1.0 / NTOT, None, OP.mult)
            msq = sp.tile([H1, 1], F32)
            nc.vector.tensor_tensor(msq[:], mean1[:], mean1[:], op=OP.mult)
            var1 = sp.tile([H1, 1], F32)
            nc.vector.tensor_tensor(var1[:], ex2[:], msq[:], op=OP.subtract)
            sd1 = sp.tile([H1, 1], F32)
            nc.scalar.activation(sd1[:], var1[:], AF.Sqrt, bias=epsc[0:H1, 0:1], scale=1.0)
            rsd1 = sp.tile([H1, 1], F32)
            nc.vector.reciprocal(rsd1[:], sd1[:])
            s1 = sp.tile([H1, 1], F32)
            nc.vector.tensor_tensor(s1[:], g1[:], rsd1[:], op=OP.mult)
            ms1 = sp.tile([H1, 1], F32)
            nc.vector.tensor_tensor(ms1[:], mean1[:], s1[:], op=OP.mult)
            t1 = sp.tile([H1, 1], F32)
            nc.vector.tensor_tensor(t1[:], be1[:], ms1[:], op=OP.subtract)

            # ================= Phase B =================
            zc = sp.tile([104, 1], F32)
            nc.vector.memset(zc[:], 0.0)
            with tc.tile_pool(name="psB", bufs=3, space="PSUM") as psB:
                for blk in range(NSG):          # 25 blocks of 2048 rows = 2 pairs
                    sl = slice(blk * SG_SL, (blk + 1) * SG_SL)
                    p1 = wp_pool.tile([H1, SG_SL], F16, name="p1", tag="actout")
                    nc.scalar.activation(p1[:], xb[:, sl], AF.Sigmoid,
                                         bias=t1[:, 0:1], scale=s1[:, 0:1])
                    if alpha1_nz:
                        nc.vector.tensor_scalar(p1[:], p1[:], am1[:, 0:1],
                                                am1[:, 1:2], OP.mult, OP.add)
                    h1 = wp_pool.tile([H1, SG_SL], F16, name="h1", tag="gpout", bufs=3)
                    SPL = 1536
                    nc.vector.tensor_tensor(
                        h1[:, 0:SPL], xb[:, blk * SG_SL:blk * SG_SL + SPL],
                        p1[:, 0:SPL], op=OP.mult)
                    nc.gpsimd.tensor_tensor(
                        h1[:, SPL:SG_SL], xb[:, blk * SG_SL + SPL:(blk + 1) * SG_SL],
                        p1[:, SPL:SG_SL], op=OP.mult)
                    for pp in range(2):         # 2 chunk-pairs per block
                        p = blk * 2 + pp
                        x2p = psB.tile([104, CH], F32, name="x2p")
                        c0 = pp * 2 * CH
                        nc.tensor.matmul(x2p[0:64, :], w2p[:],
                                         h1[:, c0:c0 + CH], start=True, stop=True)
                        nc.tensor.matmul(x2p[64:104, :], w2p[:, 0:H2],
                                         h1[:, c0 + CH:c0 + 2 * CH], start=True,
                                         stop=True, tile_position=(0, 64))
                        if pp == 0:
                            nc.scalar.copy(x2s[:, p * CH:(p + 1) * CH], x2p[:])
                        else:
                            nc.vector.tensor_tensor(
                                x2s[:, p * CH:(p + 1) * CH], x2p[:],
                                zc[:].broadcast_to([104, CH]), op=OP.add)
                        nc.vector.bn_stats(bns[:, p * 6:(p + 1) * 6],
                                           x2s[:, p * CH:(p + 1) * CH])
                        if p == 40:
                            bnaa = sp.tile([104, 2], F32, name="bnaa")
                            nc.vector.bn_aggr(bnaa[:], bns[:, 0:246])
                            sna = sp.tile([104, 2], F32, name="sna")
                            nc.vector.tensor_scalar(sna[:, 0:1], bnaa[:, 0:1],
                                                    float(41 * CH), None, OP.mult)
                            mqa = sp.tile([104, 1], F32, name="mqa")
                            nc.vector.tensor_tensor(mqa[:], bnaa[:, 0:1],
                                                    bnaa[:, 0:1], op=OP.mult)
                            nc.vector.tensor_tensor(mqa[:], bnaa[:, 1:2], mqa[:],
                                                    op=OP.add)
                            nc.vector.tensor_scalar(sna[:, 1:2], mqa[:],
                                                    float(41 * CH), None, OP.mult)
                            nc.sync.dma_start(ar2a_in[:, :], sna[:])
                            nc.gpsimd.collective_compute(
                                "AllReduce", OP.add,
                                replica_groups=[list(range(M))],
                                ins=[ar2a_in[:, :]], outs=[ar2a_out[:, :]])

            # ---- AR2b: layer-2 stats (second half) ----
            bna = sp.tile([104, 2], F32)
            nc.vector.bn_aggr(bna[:], bns[:, 246:300])
            # convert (mean, var) -> (sum, sumsq) scaled by local count R
            s2s = sp.tile([104, 2], F32)
            nc.vector.tensor_scalar(s2s[:, 0:1], bna[:, 0:1], float(9 * CH), None, OP.mult)
            m2sq = sp.tile([104, 1], F32)
            nc.vector.tensor_tensor(m2sq[:], bna[:, 0:1], bna[:, 0:1], op=OP.mult)
            nc.vector.tensor_tensor(m2sq[:], bna[:, 1:2], m2sq[:], op=OP.add)
            nc.vector.tensor_scalar(s2s[:, 1:2], m2sq[:], float(9 * CH), None, OP.mult)
            nc.sync.dma_start(ar2_in[:, :], s2s[:])
            nc.gpsimd.collective_compute(
                "AllReduce", OP.add, replica_groups=[list(range(M))],
                ins=[ar2_in[:, :]], outs=[ar2_out[:, :]])
            s2a = sp.tile([104, 2], F32)
            nc.sync.dma_start(s2a[:], ar2_out[:, :])
            s2aa = sp.tile([104, 2], F32)
            nc.sync.dma_start(s2aa[:], ar2a_out[:, :])
            nc.vector.tensor_tensor(s2a[:], s2a[:], s2aa[:], op=OP.add)
            # combine even-chunk (rows 0:40) and odd-chunk (rows 64:104) halves
            s2sw = sp.tile([104, 2], F32)
            nc.vector.memset(s2sw[:], 0.0)
            nc.sync.dma_start(s2sw[0:H2, :], s2a[64:104, :])
            nc.sync.dma_start(s2sw[64:104, :], s2a[0:H2, :])
            nc.vector.tensor_tensor(s2a[:], s2a[:], s2sw[:], op=OP.add)

            mean2 = sp.tile([104, 1], F32)
            nc.vector.tensor_scalar(mean2[:], s2a[:, 0:1], 1.0 / NTOT, None, OP.mult)
            ex22 = sp.tile([104, 1], F32)
            nc.vector.tensor_scalar(ex22[:], s2a[:, 1:2], 1.0 / NTOT, None, OP.mult)
            msq2 = sp.tile([104, 1], F32)
            nc.vector.tensor_tensor(msq2[:], mean2[:], mean2[:], op=OP.mult)
            var2 = sp.tile([104, 1], F32)
            nc.vector.tensor_tensor(var2[:], ex22[:], msq2[:], op=OP.subtract)
            sd2 = sp.tile([104, 1], F32)
            nc.scalar.activation(sd2[:], var2[:], AF.Sqrt, bias=epsc[:, 0:1], scale=1.0)
            rsd2 = sp.tile([104, 1], F32)
            nc.vector.reciprocal(rsd2[:], sd2[:])
            s2 = sp.tile([104, 1], F32)
            nc.vector.tensor_tensor(s2[:], g2[:], rsd2[:], op=OP.mult)
            ms2 = sp.tile([104, 1], F32)
            nc.vector.tensor_tensor(ms2[:], mean2[:], s2[:], op=OP.mult)
            t2 = sp.tile([104, 1], F32)
            nc.vector.tensor_tensor(t2[:], be2[:], ms2[:], op=OP.subtract)

            # ================= Phase C =================
            with (
                tc.tile_pool(name="psScore", bufs=1, space="PSUM") as psS,
                tc.tile_pool(name="psT", bufs=1, space="PSUM") as psT,
                tc.tile_pool(name="psOut", bufs=1, space="PSUM") as psO,
                tc.tile_pool(name="knr", bufs=3) as knr,
                tc.tile_pool(name="smx", bufs=2) as smx,
            ):
                # h2' slices (sigmoid2 + gate mul), stream into ring
                h2ring = []
                nco = 0
                while nco < RP:
                    w_sl = min(C_SL, RP - nco)
                    sl = slice(nco, nco + w_sl)
                    p2 = wp_pool.tile([104, C_SL], F16, name="p2", tag="actout")
                    nc.scalar.activation(p2[:, 0:w_sl], x2s[:, sl], AF.Sigmoid,
                                         bias=t2[:, 0:1], scale=s2[:, 0:1])
                    if alpha2_nz:
                        nc.vector.tensor_scalar(p2[:, 0:w_sl], p2[:, 0:w_sl],
                                                am2[:, 0:1], am2[:, 1:2],
                                                OP.mult, OP.add)
                    if b2_nz:
                        nc.vector.tensor_scalar(x2s[:, sl], x2s[:, sl],
                                                b2c[:, 0:1], None, OP.add)
                    h2 = wp_pool.tile([104, C_SL], F16, name="h2", tag="gpout", bufs=3)
                    nc.vector.tensor_tensor(h2[:, 0:w_sl], x2s[:, sl], p2[:, 0:w_sl],
                                            op=OP.mult)
                    h2ring.append((nco, w_sl, h2))
                    nco += w_sl

                def h2_slice(col, width):
                    for base, w_sl, t in h2ring:
                        if base <= col and col + width <= base + w_sl:
                            return t[:, col - base:col - base + width]
                    raise AssertionError("h2 slice spans tiles")

                score_ps = [psS.tile([128, 200], F32, name=f"score{g}")
                            for g in range(2)]
                # mm3: paired scores (s, s+2) via dual-column moving operand
                for pgrp in range(S // 4):
                    for sl4 in range(2):
                        cbase = pgrp * CH + sl4 * BSH
                        for g in range(2):
                            st = h2_slice(cbase + g * 128, 128)
                            s0 = pgrp * 4 + sl4
                            nc.tensor.matmul(
                                score_ps[g][:, s0:s0 + 3:2], st, wp2c[:],
                                start=True, stop=True)

                outp = psO.tile([128, BSH], F32)
                outs = smx.tile([64, BSH], F32, name="outs", bufs=1)
                for g in range(2):
                    # softmax over s for 128 batches
                    nmx = smx.tile([128, 1], F32, name="nmx")
                    nc.vector.tensor_reduce(nmx[:], score_ps[g][:], op=OP.max,
                                            axis=mybir.AxisListType.X, negate=True)
                    ex = smx.tile([128, 200], F32, name="ex")
                    se = smx.tile([128, 1], F32, name="se")
                    nc.scalar.activation(ex[:], score_ps[g][:], AF.Exp,
                                         bias=nmx[:, 0:1], scale=1.0,
                                         accum_out=se[:, 0:1])
                    rse = smx.tile([128, 1], F32, name="rse")
                    nc.vector.reciprocal(rse[:], se[:])
                    wgt = smx.tile([128, 200], F16, name="wgt")
                    nc.vector.tensor_scalar(wgt[:], ex[:], rse[:, 0:1], None, OP.mult)
                    # transpose w -> [s, b]
                    wta_p = psT.tile([128, 128], F16, name="wta_p")
                    nc.tensor.transpose(wta_p[:], wgt[:, 0:128], iden[:])
                    wtb_p = psT.tile([72, 128], F16, name="wtb_p")
                    nc.tensor.transpose(wtb_p[:], wgt[:, 128:200], iden[:])
                    wta = smx.tile([128, 128], F16, name="wta")
                    nc.scalar.copy(wta[:], wta_p[:])
                    wtb = smx.tile([72, 128], F16, name="wtb")
                    nc.scalar.copy(wtb[:], wtb_p[:])
                    # einsum per batch-pair: stationary [128s, 128] covers two
                    # batches; moving 2 w-cols; valid rows: 0:64 even col,
                    # 64:128 odd col
                    for bb in range(0, 128, KNB):
                        kt1 = knr.tile([128, KNB * 64], F16, name="kt1")
                        gb = g * 128 + bb
                        nc.sync.dma_start(kt1[:], kn1_d[:, gb * 64:(gb + KNB) * 64])
                        kt2 = knr.tile([72, KNB * 64], F16, name="kt2")
                        nc.sync.dma_start(kt2[:], kn2_d[:, gb * 64:(gb + KNB) * 64])
                        for ti in range(KNB // 2):
                            bcol = g * 128 + bb + 2 * ti
                            nc.tensor.matmul(
                                outp[:, bcol:bcol + 2],
                                kt1[:, ti * 128:(ti + 1) * 128],
                                wta[:, bb + 2 * ti:bb + 2 * ti + 2],
                                start=True, stop=False)
                            nc.tensor.matmul(
                                outp[:, bcol:bcol + 2],
                                kt2[:, ti * 128:(ti + 1) * 128],
                                wtb[:, bb + 2 * ti:bb + 2 * ti + 2],
                                start=False, stop=True)
                    nc.scalar.copy(
                        outs[:].rearrange("p (c two) -> p c two", two=2)
                            [:, g * 64:(g + 1) * 64, 0],
                        outp[0:64, g * 128:(g + 1) * 128:2])
                    nc.scalar.copy(
                        outs[:].rearrange("p (c two) -> p c two", two=2)
                            [:, g * 64:(g + 1) * 64, 1],
                        outp[64:128, g * 128 + 1:(g + 1) * 128:2])
                nc.sync.dma_start(out_d[:, :], outs[:])

    nc.compile()
    return nc


def _prep_inputs(query, keys, W1, b1, gamma1, beta1, alpha1,
                 W2, b2, gamma2, beta2, alpha2, Wp, bp):
    f32 = np.float32
    query = np.asarray(query, f32)
    keys = np.asarray(keys, f32)
    W1 = np.asarray(W1, f32); b1 = np.asarray(b1, f32)
    W2 = np.asarray(W2, f32); b2 = np.asarray(b2, f32)
    Wp = np.asarray(Wp, f32)

    W1a, W1b, W1c, W1d = W1[0:64], W1[64:128], W1[128:192], W1[192:256]
    w1f = np.concatenate([W1b - W1c, W1d], axis=0).astype(np.float16)  # [128, 80]

    q2 = query[:, 0, :]                                  # [B, 64]
    # global mean of xb (exact, fp32)
    mk = keys.reshape(-1, E).mean(0)                     # [64]
    mqk = (keys * query).reshape(-1, E).mean(0)          # [64]
    mu_u = (q2 @ (W1a + W1c) + b1).mean(0)               # [80]
    mean1 = ((W1b - W1c).T @ mk + W1d.T @ mqk + mu_u).astype(f32)

    w2p = np.zeros((H1, 64), np.float16)
    w2p[:, 0:H2] = W2.astype(np.float16)

    wp104 = np.zeros((104, 1), np.float16)
    wp104[0:H2, 0] = Wp[:, 0].astype(np.float16)
    wp104[64:104, 0] = Wp[:, 0].astype(np.float16)
    wp2c = np.zeros((104, 2), np.float16)
    wp2c[0:H2, 0] = Wp[:, 0].astype(np.float16)
    wp2c[64:104, 1] = Wp[:, 0].astype(np.float16)

    def pad104(v, fill):
        out = np.full((104, 1), fill, f32)
        out[0:H2, 0] = v
        out[64:104, 0] = v
        return out

    g2c = pad104(np.asarray(gamma2, f32), 1.0)
    be2c = pad104(np.asarray(beta2, f32), 0.0)
    b2c = pad104(b2, 0.0)
    am2 = np.concatenate([pad104(1.0 - np.asarray(alpha2, f32), 1.0),
                          pad104(np.asarray(alpha2, f32), 0.0)], axis=1)
    am1 = np.stack([1.0 - np.asarray(alpha1, f32), np.asarray(alpha1, f32)],
                   axis=1).astype(f32)

    iden = np.eye(128, dtype=np.float16)

    in_maps = []
    for m in range(M):
        bm = slice(m * BSH, (m + 1) * BSH)
        k_sh = keys[bm]                                  # [256, 200, 64]
        q_sh = q2[bm]                                    # [256, 64]
        kT = np.ascontiguousarray(k_sh.transpose(2, 1, 0).reshape(E, R))
        qkT = np.ascontiguousarray(
            (k_sh * q_sh[:, None, :]).transpose(2, 1, 0).reshape(E, R))
        mov = np.concatenate([kT, qkT], axis=0).astype(np.float16)
        u = np.ascontiguousarray((q_sh @ (W1a + W1c) + b1).T).astype(f32)
        ks = k_sh.transpose(1, 0, 2)                     # [200, 256, 64]
        kn1 = np.ascontiguousarray(ks[0:128].reshape(128, BSH * 64)).astype(np.float16)
        kn2 = np.ascontiguousarray(ks[128:200].reshape(72, BSH * 64)).astype(np.float16)
        in_maps.append(dict(
            mov=mov, w1f=w1f, u=u, mean1=mean1.reshape(H1, 1),
            g1=np.asarray(gamma1, f32).reshape(H1, 1),
            be1=np.asarray(beta1, f32).reshape(H1, 1),
            am1=am1, w2p=w2p, g2=g2c, be2=be2c, am2=am2, b2c=b2c,
            wp=wp104, wp2c=wp2c, kn1=kn1, kn2=kn2, iden=iden,
        ))
    flags = (bool(np.any(np.asarray(alpha1))), bool(np.any(np.asarray(alpha2))),
             bool(np.any(np.asarray(b2))))
    return in_maps, flags


def kernel(**inputs):
    in_maps, flags = _prep_inputs(**inputs)
    if flags not in _CACHE:
        _CACHE[flags] = _build(*flags)
    nc = _CACHE[flags]
    res = run_bass_kernel_spmd(nc, in_maps, core_ids=list(range(M)))
    outs = [res.results[m]["out"].T for m in range(M)]   # [256, 64] each
    return np.concatenate(outs, axis=0).astype(np.float32)


if __name__ == "__main__":
    rng = np.random.default_rng(0)
    pass
